# revision 13
# baseline (speedup 1.0000x reference)
"""CfC head (3 stacked CfC cells, seq_len=1, h0=0) on 8 TRN2 NeuronCores.

Math (per cell, zero initial hidden state, ts=1):
    ff1 = tanh(x @ (Wf1*mask)[:in] + bf1)
    ff2 = tanh(x @ (Wf2*mask)[:in] + bf2)
    s   = sigmoid(x @ (Wtb - Wta)[:in] + (btb - bta))
    out = ff1 + s * (ff2 - ff1)

Because h0 == 0 only the first in_dim rows of each weight matter, the
sparsity mask folds into the weights, and t_a/t_b fold into a single
matmul.  That is all O(params) host-side prep; the O(B) work runs on
the NeuronCores, data-parallel over the batch.

Device layout is feature-major ([feat, batch]); x is transposed on the
host so every DMA is contiguous per partition.  Per 2048-column slice,
each layer is computed as a set of "instances":
  - full 128-row M-tiles, processed in two 1024-column halves
    (PSUM tiles are [128,1024] = 2 banks, 4 pool slots -> deep
    PE/ACT overlap), and
  - the M-tail (hid % 128 = 13/51/64 rows), batch-STACKED across
    column-tile positions of the PE array: 4 (or 2) batch chunks are
    packed onto partition groups at 32/64-partition offsets via
    tile_position, so the tail costs one narrow ACT/DVE op instead of
    four wide ones, and its matmuls run concurrently on distinct PE
    column groups.
The next layer contracts over the previous outputs in place: stacked
tail rows are consumed by row-positioned (tile_position) K-tail
matmuls against weight tiles replicated at the matching partition
offsets.
"""

import numpy as np

import concourse.bass as bass
import concourse.tile as tile
from concourse import mybir
from concourse.bass_utils import run_bass_kernel_spmd

# ---------------------------------------------------------------- dims
INPUT_DIM, INTER, COMMAND, MOTOR = 74, 269, 179, 64
# L0's contraction is zero-padded 74 -> 128: the PE activity monitor
# only counts full-row (K=128) matmuls, so K=74 work runs at the cold
# 1.2 GHz clock forever.  Full-K L0 matmuls keep the PE at 2.4 GHz.
K0_PAD = 128
BATCH = 65536
N_CORES = 8
B_CORE = BATCH // N_CORES          # 8192 rows per core
G = 2048                           # batch columns per pipeline slice
NCH = G // 512                     # 512-column matmul chunks per slice
SLICES = B_CORE // G

LAYER_DIMS = [(INPUT_DIM, INTER), (INTER, COMMAND), (COMMAND, MOTOR)]
MATS = ("f1", "f2", "t")
F32 = mybir.dt.float32
F32R = mybir.dt.float32r
# matmul operand / activation storage dtype.  bf16 allows PE column
# tile_position (stacked tails) and fast weight load; PSUM accumulation
# stays fp32.  (f32r rejects column tile positions in this walrus
# build.)
MM_DT = mybir.dt.bfloat16
# lerp elementwise dtype: fp16 keeps 10 mantissa bits and runs the DVE
# tensor_tensor ops in the 2x 16-bit mode.
EW_DT = mybir.dt.float16


def _tail_spec(hid):
    """(n_full, r, stride, ngroups, cpg) for a layer's M dimension."""
    n_full = hid // 128
    r = hid - 128 * n_full
    if r == 0:
        return n_full, 0, 0, 0, 0
    stride = 32 if r <= 32 else 64
    ngroups = 128 // stride
    cpg = NCH // ngroups           # batch chunks stacked per partition group
    return n_full, r, stride, ngroups, cpg


def _instances(hid):
    # tail first: its output gates the next layer's K-tail matmuls, so
    # let its (cheap) elementwise chain run under the fulls' matmuls
    n_full, r, stride, ngroups, cpg = _tail_spec(hid)
    out = []
    if r:
        out.append(("tail",))
    for h in range(NCH // 2):
        for mi in range(n_full):
            out.append(("full", mi, h))
    return out


# bias pack columns: one per (layer, mat, m-range); full halves share.
BIAS_COLS = {}
_c = 0
for _l, (_in, _hid) in enumerate(LAYER_DIMS):
    _nf, _r, _st, _ng, _cpg = _tail_spec(_hid)
    for _mat in MATS:
        for _mi in range(_nf):
            BIAS_COLS[(_l, _mat, "full", _mi)] = _c
            _c += 1
        if _r:
            BIAS_COLS[(_l, _mat, "tail")] = _c
            _c += 1
N_BIAS_COLS = _c


def _mranges(l):
    # output m-ranges; tail m-size padded to the group stride with zero
    # columns so stacked PSUM groups cover every partition
    nf, r, st, ng, cpg = _tail_spec(LAYER_DIMS[l][1])
    out = [("full", mi, mi * 128, 128) for mi in range(nf)]
    if r:
        out.append(("tail", None, nf * 128, st))
    return out


def _in_kparts(l):
    if l == 0:
        return [("main", 0, 0, K0_PAD)]
    nf, r, st, ng, cpg = _tail_spec(LAYER_DIMS[l - 1][1])
    parts = [("main", ki, ki * 128, 128) for ki in range(nf)]
    if r:
        parts.append(("ktail", None, nf * 128, r))
    return parts


def _wpack_layout():
    # all lhsT tiles as column blocks of one [128, NW] array; k-tail
    # blocks carry weight rows replicated at each stacked group base
    cols = {}
    c = 0
    for l in range(len(LAYER_DIMS)):
        for mat in MATS:
            for mkind, mi, m0, msz in _mranges(l):
                for kkind, ki, k0, ksz in _in_kparts(l):
                    cols[(l, mat, mkind, mi, kkind, ki)] = (c, msz)
                    c += msz
    return cols, c


WPACK_COLS, NW = _wpack_layout()


# ---------------------------------------------- walrus sync-wait workaround
def _split_multi_waits(nc):
    """This walrus build accepts only ONE sync-wait command per
    instruction.  Tile attaches one wait per outstanding proc, so after
    scheduling, hoist every excess wait onto a single-wait NOP emitted
    just before the instruction on the same engine (engine queues are
    in-order, so the waits still all complete before it executes)."""
    import bass_rust as _br

    for fn in nc.m.functions:
        for blk in fn.blocks:
            out = []
            changed = False
            for inst in blk.instructions:
                si = inst.sync_info
                if si is not None and len(si.on_wait) > 1:
                    waits = list(si.on_wait)
                    for j, w in enumerate(waits[:-1]):
                        carrier = mybir.InstNoOp(
                            name=f"{inst.name}-sw{j}", engine=inst.engine
                        )
                        carrier.sync_info = _br.SyncInfo(on_wait=[w], on_update=[])
                        out.append(carrier)
                    inst.sync_info = _br.SyncInfo(
                        on_wait=[waits[-1]], on_update=list(si.on_update)
                    )
                    changed = True
                out.append(inst)
            if changed:
                blk.instructions = out
    return nc


# ---------------------------------------------------------------- device
class _LayerOut:
    """Feature-major activation of one layer for one slice.

    halves: (mi, h) -> [128, 1024] tile (feature rows mi*128..+128,
            batch chunks 2h, 2h+1).
    tail:   [stride*(ngroups-1)+r, 512*cpg] tile; partition group g
            holds feature rows n_full*128..+r for batch chunks
            g*cpg..(g+1)*cpg.
    """

    def __init__(self, hid):
        self.hid = hid
        self.n_full, self.r, self.stride, self.ngroups, self.cpg = _tail_spec(hid)
        self.halves = {}
        self.tail = None

    def kparts(self):
        parts = [("main", ki, ki * 128, 128) for ki in range(self.n_full)]
        if self.r:
            parts.append(("ktail", None, self.n_full * 128, self.r))
        return parts

    def rhs(self, kind, ki, c):
        """(ap, row_pos) of this output as contraction input, chunk c."""
        if kind == "main":
            t = self.halves[(ki, c // 2)]
            f0 = (c % 2) * 512
            return t[:, f0 : f0 + 512], 0
        g = c // self.cpg
        p0 = self.stride * g
        f0 = (c % self.cpg) * 512
        return self.tail[p0 : p0 + self.r, f0 : f0 + 512], p0


def _build_nc(repeat=1):
    nc = bass.Bass(target_bir_lowering=False)

    xT = nc.dram_tensor("xT", [K0_PAD, B_CORE], MM_DT, kind="ExternalInput")
    wpack_dram = nc.dram_tensor("wpack", [128, NW], MM_DT, kind="ExternalInput")
    bias_dram = nc.dram_tensor("biases", [128, N_BIAS_COLS], F32, kind="ExternalInput")
    # stacked output layout; host unpacks (see _unpack_out)
    outT = nc.dram_tensor("outT", [128, B_CORE // 2], F32, kind="ExternalOutput")

    TANH = mybir.ActivationFunctionType.Tanh
    SIGM = mybir.ActivationFunctionType.Sigmoid

    with tile.TileContext(nc) as tc:
        with (
            tc.tile_pool(name="consts", bufs=1) as consts,
            tc.tile_pool(name="xt", bufs=3) as xt_pool,
            tc.tile_pool(name="act", bufs=4) as act_pool,
            tc.tile_pool(name="ff", bufs=6) as ff_pool,
            tc.tile_pool(name="ps", bufs=4, space="PSUM") as ps_pool,
        ):
            # consts arrive via the ACT HWDGE ring so the SP ring can
            # start streaming x immediately
            bias_sb = consts.tile([128, N_BIAS_COLS], F32, tag="bias")
            nc.scalar.dma_start(out=bias_sb[:], in_=bias_dram[:])
            wpack_sb = consts.tile([128, NW], MM_DT, tag="wpack")
            nc.scalar.dma_start(out=wpack_sb[:], in_=wpack_dram[:])

            # PE warm-up: the HAM activity monitor only registers
            # full-row (K=128) matmul activity, so a kernel that opens
            # with K=74 L0 matmuls runs its entire span at the cold
            # 1.2 GHz clock.  A short block of full-K zero matmuls
            # (overlapping the initial DMA wait) trips the un-throttle;
            # the real work then issues at 2.4 GHz, and partial-K
            # activity keeps the PE from re-throttling.
            # ~8 MMs (3.4us) trip the un-throttle; the rest keep the PE
            # busy across the wpack/xt DMA wait (~10us) so the HAM MID
            # window (~5.2us idle) can't re-throttle before real work.
            N_WARM = 32
            warm = consts.tile([128, 640], MM_DT, tag="warm")
            nc.vector.memset(warm[:], 0.0)
            warm_ps = ps_pool.tile([128, 512], F32, tag="ps")
            for i in range(N_WARM):
                nc.tensor.matmul(
                    warm_ps[:],
                    warm[:, 0:128],
                    warm[:, 128:640],
                    start=(i == 0),
                    stop=(i == N_WARM - 1),
                )

            # ---- per-slice pipeline (repeat>1 reruns the same work for
            # differential wall-clock timing; outputs just overwritten)
            def make_xt_in(s):
                c0 = s * G
                xt = xt_pool.tile([K0_PAD, G], MM_DT, tag="xt")
                nc.sync.dma_start(out=xt[:], in_=xT[:, c0 : c0 + G])

                class _XtIn:
                    @staticmethod
                    def kparts():
                        return [("main", 0, 0, K0_PAD)]

                    @staticmethod
                    def rhs(kind, ki, c):
                        return xt[:, c * 512 : (c + 1) * 512], 0

                return _XtIn

            if True:

                def layer_tasks(l, lin, out_dtype=MM_DT):
                    """Return (lo, thunks): one thunk per instance;
                    running a thunk emits that instance's IR and files
                    its output tile into lo."""
                    lo = _LayerOut(LAYER_DIMS[l][1])
                    kps = lin.kparts()

                    def instance(inst):
                        if inst[0] == "full":
                            _, mi, h = inst
                            P, FF = 128, 1024
                            chunks = [2 * h, 2 * h + 1]
                            mkind, mmi = "full", mi

                            def region(c):
                                f0 = (c % 2) * 512
                                return slice(0, 128), slice(f0, f0 + 512), 0
                        else:
                            P = 128
                            FF = 512 * lo.cpg
                            # alternate column groups so adjacent matmuls
                            # land on distinct PE column tiles
                            chunks = sorted(
                                range(NCH), key=lambda c: (c % lo.cpg, c)
                            )
                            mkind, mmi = "tail", None

                            def region(c):
                                g = c // lo.cpg
                                p0 = lo.stride * g
                                f0 = (c % lo.cpg) * 512
                                return (
                                    slice(p0, p0 + lo.stride),
                                    slice(f0, f0 + 512),
                                    p0,
                                )

                        ff = {}
                        for mat in MATS:
                            ps = ps_pool.tile([P, FF], F32, tag="ps")
                            for c in chunks:
                                psl, fsl, colp = region(c)
                                for kpi, (kkind, ki, k0, ksz) in enumerate(kps):
                                    wc0, wmsz = WPACK_COLS[
                                        (l, mat, mkind, mmi, kkind, ki)
                                    ]
                                    rhs_ap, rowp = lin.rhs(kkind, ki, c)
                                    lhsT = wpack_sb[
                                        rowp : rowp + ksz, wc0 : wc0 + wmsz
                                    ]
                                    nc.tensor.matmul(
                                        ps[psl, fsl],
                                        lhsT,
                                        rhs_ap,
                                        start=(kpi == 0),
                                        stop=(kpi == len(kps) - 1),
                                        tile_position=(rowp, colp),
                                    )
                            f = ff_pool.tile([P, FF], EW_DT, tag=f"ff_{mat}")
                            bcol = BIAS_COLS[
                                (l, mat, "full", inst[1])
                                if inst[0] == "full"
                                else (l, mat, "tail")
                            ]
                            nc.scalar.activation(
                                out=f[:],
                                in_=ps[:],
                                func=SIGM if mat == "t" else TANH,
                                bias=bias_sb[:P, bcol : bcol + 1],
                            )
                            ff[mat] = f
                        # out = ff1 + s*(ff2-ff1); 16-bit DVE chain
                        d = ff_pool.tile([P, FF], EW_DT, tag="d")
                        nc.vector.tensor_sub(d[:], ff["f2"][:], ff["f1"][:])
                        nc.vector.tensor_mul(ff["f2"][:], ff["t"][:], d[:])
                        tag = (
                            f"o{l}_{inst[1]}_{inst[2]}"
                            if inst[0] == "full"
                            else f"o{l}_tail"
                        )
                        o = act_pool.tile([P, FF], out_dtype, tag=tag)
                        nc.vector.tensor_add(o[:], ff["f1"][:], ff["f2"][:])
                        return o

                    def run_inst(inst):
                        o = instance(inst)
                        if inst[0] == "full":
                            lo.halves[(inst[1], inst[2])] = o
                        else:
                            lo.tail = o

                    thunks = [
                        (lambda inst=inst: run_inst(inst))
                        for inst in _instances(lo.hid)
                    ]
                    return lo, thunks

            # software pipeline across layers: stage t emits L0(slice t),
            # L1(slice t-1), L2(slice t-2) with their instances
            # round-robin interleaved.  ACT-heavy L0 work then always
            # coexists with PE-heavy L1 work in the engine queues, so
            # neither engine starves (layer-major pairs left the PE
            # idle-bursting during L0 phases and the ACT idle during L1).
            total = SLICES * repeat
            lo0, lo1 = {}, {}
            for t in range(total + 2):
                lists = []
                if t < total:
                    s = t % SLICES
                    xin = make_xt_in(s)
                    lo0[t], th0 = layer_tasks(0, xin)
                    lists.append(th0)
                if 0 <= t - 1 < total:
                    lo1[t - 1], th1 = layer_tasks(1, lo0.pop(t - 1))
                    lists.append(th1)
                lo2 = None
                if 0 <= t - 2 < total:
                    lo2, th2 = layer_tasks(2, lo1.pop(t - 2), out_dtype=F32)
                    lists.append(th2)
                # round-robin: preserves each layer's internal order
                # (tail first) while mixing layers in the engine queues
                i = 0
                while any(lists):
                    if lists[i % len(lists)]:
                        lists[i % len(lists)].pop(0)()
                    i += 1
                if lo2 is not None:
                    s2 = (t - 2) % SLICES
                    # L2 output is a single stacked [128, 1024] tile
                    nc.sync.dma_start(
                        out=outT[:, s2 * 1024 : (s2 + 1) * 1024],
                        in_=lo2.tail[:],
                    )

    return nc


_NC_CACHE = {}


def _get_nc(repeat=1):
    if repeat not in _NC_CACHE:
        _NC_CACHE[repeat] = _split_multi_waits(_build_nc(repeat))
    return _NC_CACHE[repeat]


# ------------------------------------------------------------------ host
def _prep_host_inputs(inputs):
    """Fold masks / t-diff, pack biases, shard x.  Returns per-core maps."""
    f32 = np.float32
    common = {}
    folded = {}
    for l, (ind, hid) in enumerate(LAYER_DIMS):
        m = inputs[f"mask_{l}"][:ind].astype(f32)
        folded[(l, "f1")] = (inputs[f"Wf1_{l}"][:ind] * m).astype(f32)
        folded[(l, "f2")] = (inputs[f"Wf2_{l}"][:ind] * m).astype(f32)
        folded[(l, "t")] = (
            inputs[f"Wtb_{l}"][:ind] - inputs[f"Wta_{l}"][:ind]
        ).astype(f32)
    for mat in MATS:  # zero-pad L0 contraction rows 74..127
        W = folded[(0, mat)]
        folded[(0, mat)] = np.concatenate(
            [W, np.zeros((K0_PAD - W.shape[0], W.shape[1]), f32)], axis=0
        )
    wpack = np.zeros((128, NW), dtype=f32)
    for (l, mat, mkind, mi, kkind, ki), (c0, msz) in WPACK_COLS.items():
        W = folded[(l, mat)]
        _, hid = LAYER_DIMS[l]
        m0 = mi * 128 if mkind == "full" else (hid // 128) * 128
        rm = min(msz, hid - m0)
        kp = [p for p in _in_kparts(l) if p[0] == kkind and p[1] == ki][0]
        _, _, k0, ksz = kp
        if kkind == "ktail":
            pnf, pr, pst, png, pcpg = _tail_spec(LAYER_DIMS[l - 1][1])
            for g in range(png):
                wpack[pst * g : pst * g + ksz, c0 : c0 + rm] = W[
                    k0 : k0 + ksz, m0 : m0 + rm
                ]
        else:
            wpack[:ksz, c0 : c0 + rm] = W[k0 : k0 + ksz, m0 : m0 + rm]
    import ml_dtypes
    np_mm = mybir.dt.np(MM_DT)
    common["wpack"] = wpack.astype(np_mm)
    biases = np.zeros((128, N_BIAS_COLS), dtype=f32)
    for l, (ind, hid) in enumerate(LAYER_DIMS):
        n_full, r, stride, ngroups, cpg = _tail_spec(hid)
        bmats = {
            "f1": inputs[f"bf1_{l}"],
            "f2": inputs[f"bf2_{l}"],
            "t": inputs[f"btb_{l}"] - inputs[f"bta_{l}"],
        }
        for mat, b in bmats.items():
            for mi in range(n_full):
                biases[:, BIAS_COLS[(l, mat, "full", mi)]] = b[
                    mi * 128 : (mi + 1) * 128
                ]
            if r:
                col = BIAS_COLS[(l, mat, "tail")]
                for g in range(ngroups):
                    biases[g * stride : g * stride + r, col] = b[
                        n_full * 128 : n_full * 128 + r
                    ]
    common["biases"] = biases

    xT = np.zeros((K0_PAD, BATCH), dtype=np_mm)
    xT[:INPUT_DIM] = np.asarray(inputs["x"], dtype=f32).T.astype(np_mm)
    in_maps = []
    for c in range(N_CORES):
        m = dict(common)
        m["xT"] = np.ascontiguousarray(xT[:, c * B_CORE : (c + 1) * B_CORE])
        in_maps.append(m)
    return in_maps


def _unpack_out(outT_core):
    """[128, B_CORE//2] stacked -> [B_CORE, 64].

    Per slice s, column block [:, s*1024:(s+1)*1024]: rows 64g..64g+64
    hold batch chunks (2g, 2g+1) of that slice at free offsets 0/512.
    """
    out = np.empty((B_CORE, MOTOR), dtype=outT_core.dtype)
    for s in range(SLICES):
        blk = outT_core[:, s * 1024 : (s + 1) * 1024]
        for c in range(NCH):
            g, f0 = c // 2, (c % 2) * 512
            rows = slice(s * G + c * 512, s * G + (c + 1) * 512)
            out[rows, :] = blk[64 * g : 64 * g + 64, f0 : f0 + 512].T
    return out


def run(inputs, trace=False, repeat=1, **kw):
    """Run on hardware; returns (out [BATCH, MOTOR] fp32, results)."""
    nc = _get_nc(repeat)
    in_maps = _prep_host_inputs(inputs)
    res = run_bass_kernel_spmd(
        nc, in_maps, core_ids=list(range(N_CORES)), trace=trace, **kw
    )
    out = np.empty((BATCH, MOTOR), dtype=np.float32)
    for c in range(N_CORES):
        out[c * B_CORE : (c + 1) * B_CORE, :] = _unpack_out(res.results[c]["outT"])
    return out, res


def kernel(**inputs) -> np.ndarray:
    out, _ = run(inputs, trace=False)
    return out



# revision 14
# speedup vs baseline: 1.1680x; 1.1680x over previous
"""CfC head (3 stacked CfC cells, seq_len=1, h0=0) on 8 TRN2 NeuronCores.

Math (per cell, zero initial hidden state, ts=1):
    ff1 = tanh(x @ (Wf1*mask)[:in] + bf1)
    ff2 = tanh(x @ (Wf2*mask)[:in] + bf2)
    s   = sigmoid(x @ (Wtb - Wta)[:in] + (btb - bta))
    out = ff1 + s * (ff2 - ff1)

Because h0 == 0 only the first in_dim rows of each weight matter, the
sparsity mask folds into the weights, and t_a/t_b fold into a single
matmul.  That is all O(params) host-side prep; the O(B) work runs on
the NeuronCores, data-parallel over the batch.

Device layout is feature-major ([feat, batch]); x is transposed on the
host so every DMA is contiguous per partition.  Per 2048-column slice,
each layer is computed as a set of "instances":
  - full 128-row M-tiles, processed in two 1024-column halves
    (PSUM tiles are [128,1024] = 2 banks, 4 pool slots -> deep
    PE/ACT overlap), and
  - the M-tail (hid % 128 = 13/51/64 rows), batch-STACKED across
    column-tile positions of the PE array: 4 (or 2) batch chunks are
    packed onto partition groups at 32/64-partition offsets via
    tile_position, so the tail costs one narrow ACT/DVE op instead of
    four wide ones, and its matmuls run concurrently on distinct PE
    column groups.
The next layer contracts over the previous outputs in place: stacked
tail rows are consumed by row-positioned (tile_position) K-tail
matmuls against weight tiles replicated at the matching partition
offsets.
"""

import numpy as np

import concourse.bass as bass
import concourse.tile as tile
from concourse import mybir
from concourse.bass_utils import run_bass_kernel_spmd

# ---------------------------------------------------------------- dims
INPUT_DIM, INTER, COMMAND, MOTOR = 74, 269, 179, 64
# L0's contraction is zero-padded 74 -> 128: the PE activity monitor
# only counts full-row (K=128) matmuls, so K=74 work runs at the cold
# 1.2 GHz clock forever.  Full-K L0 matmuls keep the PE at 2.4 GHz.
K0_PAD = 128
BATCH = 65536
N_CORES = 8
B_CORE = BATCH // N_CORES          # 8192 rows per core
G = 2048                           # batch columns per pipeline slice
NCH = G // 512                     # 512-column matmul chunks per slice
SLICES = B_CORE // G

LAYER_DIMS = [(INPUT_DIM, INTER), (INTER, COMMAND), (COMMAND, MOTOR)]
MATS = ("f1", "f2", "t")
F32 = mybir.dt.float32
F32R = mybir.dt.float32r
# matmul operand / activation storage dtype.  bf16 allows PE column
# tile_position (stacked tails) and fast weight load; PSUM accumulation
# stays fp32.  (f32r rejects column tile positions in this walrus
# build.)
MM_DT = mybir.dt.bfloat16
# lerp elementwise dtype: fp16 keeps 10 mantissa bits and runs the DVE
# tensor_tensor ops in the 2x 16-bit mode.
EW_DT = mybir.dt.float16


def _tail_spec(hid):
    """(n_full, r, stride, ngroups, cpg) for a layer's M dimension."""
    n_full = hid // 128
    r = hid - 128 * n_full
    if r == 0:
        return n_full, 0, 0, 0, 0
    stride = 32 if r <= 32 else 64
    ngroups = 128 // stride
    cpg = NCH // ngroups           # batch chunks stacked per partition group
    return n_full, r, stride, ngroups, cpg


def _instances(hid):
    # tail first: its output gates the next layer's K-tail matmuls, so
    # let its (cheap) elementwise chain run under the fulls' matmuls
    n_full, r, stride, ngroups, cpg = _tail_spec(hid)
    out = []
    if r:
        out.append(("tail",))
    for h in range(NCH // 2):
        for mi in range(n_full):
            out.append(("full", mi, h))
    return out


# bias pack columns: one per (layer, mat, m-range); full halves share.
BIAS_COLS = {}
_c = 0
for _l, (_in, _hid) in enumerate(LAYER_DIMS):
    _nf, _r, _st, _ng, _cpg = _tail_spec(_hid)
    for _mat in MATS:
        for _mi in range(_nf):
            BIAS_COLS[(_l, _mat, "full", _mi)] = _c
            _c += 1
        if _r:
            BIAS_COLS[(_l, _mat, "tail")] = _c
            _c += 1
N_BIAS_COLS = _c


def _mranges(l):
    # output m-ranges; tail m-size padded to the group stride with zero
    # columns so stacked PSUM groups cover every partition
    nf, r, st, ng, cpg = _tail_spec(LAYER_DIMS[l][1])
    out = [("full", mi, mi * 128, 128) for mi in range(nf)]
    if r:
        out.append(("tail", None, nf * 128, st))
    return out


def _in_kparts(l):
    if l == 0:
        return [("main", 0, 0, K0_PAD)]
    nf, r, st, ng, cpg = _tail_spec(LAYER_DIMS[l - 1][1])
    parts = [("main", ki, ki * 128, 128) for ki in range(nf)]
    if r:
        parts.append(("ktail", None, nf * 128, r))
    return parts


def _wpack_layout():
    # all lhsT tiles as column blocks of one [128, NW] array; k-tail
    # blocks carry weight rows replicated at each stacked group base
    cols = {}
    c = 0
    for l in range(len(LAYER_DIMS)):
        for mat in MATS:
            for mkind, mi, m0, msz in _mranges(l):
                for kkind, ki, k0, ksz in _in_kparts(l):
                    cols[(l, mat, mkind, mi, kkind, ki)] = (c, msz)
                    c += msz
    return cols, c


WPACK_COLS, NW = _wpack_layout()


# ---------------------------------------------- walrus sync-wait workaround
def _split_multi_waits(nc):
    """This walrus build accepts only ONE sync-wait command per
    instruction.  Tile attaches one wait per outstanding proc, so after
    scheduling, hoist every excess wait onto a single-wait NOP emitted
    just before the instruction on the same engine (engine queues are
    in-order, so the waits still all complete before it executes)."""
    import bass_rust as _br

    for fn in nc.m.functions:
        for blk in fn.blocks:
            out = []
            changed = False
            for inst in blk.instructions:
                si = inst.sync_info
                if si is not None and len(si.on_wait) > 1:
                    waits = list(si.on_wait)
                    for j, w in enumerate(waits[:-1]):
                        carrier = mybir.InstNoOp(
                            name=f"{inst.name}-sw{j}", engine=inst.engine
                        )
                        carrier.sync_info = _br.SyncInfo(on_wait=[w], on_update=[])
                        out.append(carrier)
                    inst.sync_info = _br.SyncInfo(
                        on_wait=[waits[-1]], on_update=list(si.on_update)
                    )
                    changed = True
                out.append(inst)
            if changed:
                blk.instructions = out
    return nc


# ---------------------------------------------------------------- device
class _LayerOut:
    """Feature-major activation of one layer for one slice.

    halves: (mi, h) -> [128, 1024] tile (feature rows mi*128..+128,
            batch chunks 2h, 2h+1).
    tail:   [stride*(ngroups-1)+r, 512*cpg] tile; partition group g
            holds feature rows n_full*128..+r for batch chunks
            g*cpg..(g+1)*cpg.
    """

    def __init__(self, hid):
        self.hid = hid
        self.n_full, self.r, self.stride, self.ngroups, self.cpg = _tail_spec(hid)
        self.halves = {}
        self.tail = None

    def kparts(self):
        parts = [("main", ki, ki * 128, 128) for ki in range(self.n_full)]
        if self.r:
            parts.append(("ktail", None, self.n_full * 128, self.r))
        return parts

    def rhs(self, kind, ki, c):
        """(ap, row_pos) of this output as contraction input, chunk c."""
        if kind == "main":
            t = self.halves[(ki, c // 2)]
            f0 = (c % 2) * 512
            return t[:, f0 : f0 + 512], 0
        g = c // self.cpg
        p0 = self.stride * g
        f0 = (c % self.cpg) * 512
        return self.tail[p0 : p0 + self.r, f0 : f0 + 512], p0


def _build_nc(repeat=1):
    nc = bass.Bass(target_bir_lowering=False)

    xT = nc.dram_tensor("xT", [K0_PAD, B_CORE], MM_DT, kind="ExternalInput")
    wpack_dram = nc.dram_tensor("wpack", [128, NW], MM_DT, kind="ExternalInput")
    bias_dram = nc.dram_tensor("biases", [128, N_BIAS_COLS], F32, kind="ExternalInput")
    # stacked output layout; host unpacks (see _unpack_out)
    outT = nc.dram_tensor("outT", [128, B_CORE // 2], F32, kind="ExternalOutput")

    TANH = mybir.ActivationFunctionType.Tanh
    SIGM = mybir.ActivationFunctionType.Sigmoid

    with tile.TileContext(nc) as tc:
        with (
            tc.tile_pool(name="consts", bufs=1) as consts,
            tc.tile_pool(name="xt", bufs=3) as xt_pool,
            tc.tile_pool(name="act", bufs=4) as act_pool,
            tc.tile_pool(name="ff", bufs=6) as ff_pool,
            tc.tile_pool(name="ps", bufs=4, space="PSUM") as ps_pool,
        ):
            # consts arrive via the ACT HWDGE ring so the SP ring can
            # start streaming x immediately
            bias_sb = consts.tile([128, N_BIAS_COLS], F32, tag="bias")
            nc.scalar.dma_start(out=bias_sb[:], in_=bias_dram[:])
            wpack_sb = consts.tile([128, NW], MM_DT, tag="wpack")
            nc.scalar.dma_start(out=wpack_sb[:], in_=wpack_dram[:])

            # PE warm-up: the HAM activity monitor only registers
            # full-row (K=128) matmul activity, so a kernel that opens
            # with K=74 L0 matmuls runs its entire span at the cold
            # 1.2 GHz clock.  A short block of full-K zero matmuls
            # (overlapping the initial DMA wait) trips the un-throttle;
            # the real work then issues at 2.4 GHz, and partial-K
            # activity keeps the PE from re-throttling.
            # ~8 MMs (3.4us) trip the un-throttle; the rest keep the PE
            # busy across the wpack/xt DMA wait (~10us) so the HAM MID
            # window (~5.2us idle) can't re-throttle before real work.
            N_WARM = 32
            warm = consts.tile([128, 640], MM_DT, tag="warm")
            nc.vector.memset(warm[:], 0.0)
            warm_ps = ps_pool.tile([128, 512], F32, tag="ps")
            for i in range(N_WARM):
                nc.tensor.matmul(
                    warm_ps[:],
                    warm[:, 0:128],
                    warm[:, 128:640],
                    start=(i == 0),
                    stop=(i == N_WARM - 1),
                )

            # ---- per-slice pipeline (repeat>1 reruns the same work for
            # differential wall-clock timing; outputs just overwritten)
            def make_xt_in(s):
                c0 = s * G
                xt = xt_pool.tile([K0_PAD, G], MM_DT, tag="xt")
                nc.sync.dma_start(out=xt[:], in_=xT[:, c0 : c0 + G])

                class _XtIn:
                    @staticmethod
                    def kparts():
                        return [("main", 0, 0, K0_PAD)]

                    @staticmethod
                    def rhs(kind, ki, c):
                        return xt[:, c * 512 : (c + 1) * 512], 0

                return _XtIn

            if True:

                def layer_tasks(l, lin, out_dtype=MM_DT):
                    """Return (lo, thunks): one thunk per instance;
                    running a thunk emits that instance's IR and files
                    its output tile into lo."""
                    lo = _LayerOut(LAYER_DIMS[l][1])
                    kps = lin.kparts()

                    def instance(inst):
                        if inst[0] == "full":
                            _, mi, h = inst
                            P, FF = 128, 1024
                            chunks = [2 * h, 2 * h + 1]
                            mkind, mmi = "full", mi

                            def region(c):
                                f0 = (c % 2) * 512
                                return slice(0, 128), slice(f0, f0 + 512), 0
                        else:
                            P = 128
                            FF = 512 * lo.cpg
                            # alternate column groups so adjacent matmuls
                            # land on distinct PE column tiles
                            chunks = sorted(
                                range(NCH), key=lambda c: (c % lo.cpg, c)
                            )
                            mkind, mmi = "tail", None

                            def region(c):
                                g = c // lo.cpg
                                p0 = lo.stride * g
                                f0 = (c % lo.cpg) * 512
                                return (
                                    slice(p0, p0 + lo.stride),
                                    slice(f0, f0 + 512),
                                    p0,
                                )

                        ff = {}
                        for mat in MATS:
                            ps = ps_pool.tile([P, FF], F32, tag="ps")
                            for c in chunks:
                                psl, fsl, colp = region(c)
                                for kpi, (kkind, ki, k0, ksz) in enumerate(kps):
                                    wc0, wmsz = WPACK_COLS[
                                        (l, mat, mkind, mmi, kkind, ki)
                                    ]
                                    rhs_ap, rowp = lin.rhs(kkind, ki, c)
                                    lhsT = wpack_sb[
                                        rowp : rowp + ksz, wc0 : wc0 + wmsz
                                    ]
                                    nc.tensor.matmul(
                                        ps[psl, fsl],
                                        lhsT,
                                        rhs_ap,
                                        start=(kpi == 0),
                                        stop=(kpi == len(kps) - 1),
                                        tile_position=(rowp, colp),
                                    )
                            f = ff_pool.tile([P, FF], EW_DT, tag=f"ff_{mat}")
                            bcol = BIAS_COLS[
                                (l, mat, "full", inst[1])
                                if inst[0] == "full"
                                else (l, mat, "tail")
                            ]
                            nc.scalar.activation(
                                out=f[:],
                                in_=ps[:],
                                func=SIGM if mat == "t" else TANH,
                                bias=bias_sb[:P, bcol : bcol + 1],
                            )
                            ff[mat] = f
                        # out = ff1 + s*(ff2-ff1); 16-bit DVE chain
                        d = ff_pool.tile([P, FF], EW_DT, tag="d")
                        nc.vector.tensor_sub(d[:], ff["f2"][:], ff["f1"][:])
                        nc.vector.tensor_mul(ff["f2"][:], ff["t"][:], d[:])
                        tag = (
                            f"o{l}_{inst[1]}_{inst[2]}"
                            if inst[0] == "full"
                            else f"o{l}_tail"
                        )
                        o = act_pool.tile([P, FF], out_dtype, tag=tag)
                        nc.vector.tensor_add(o[:], ff["f1"][:], ff["f2"][:])
                        return o

                    def run_inst(inst):
                        o = instance(inst)
                        if inst[0] == "full":
                            lo.halves[(inst[1], inst[2])] = o
                        else:
                            lo.tail = o

                    thunks = [
                        (lambda inst=inst: run_inst(inst))
                        for inst in _instances(lo.hid)
                    ]
                    return lo, thunks

            # process slices in pairs, layer-major: dense same-layer
            # matmul streams keep the PE activity monitor warm (mixing
            # layers dilutes the full-K density and re-throttles the
            # PE); each layer's fill-latency overlaps the sibling
            # slice's dense work
            PAIR = 2
            total = SLICES * repeat

            def run_layer(l, lin, out_dtype=MM_DT):
                lo, thunks = layer_tasks(l, lin, out_dtype)
                for th in thunks:
                    th()
                return lo

            for pr in range(0, total, PAIR):
                sl = [(pr + j) % SLICES for j in range(min(PAIR, total - pr))]
                outs = [make_xt_in(s) for s in sl]
                outs = [run_layer(0, x) for x in outs]
                outs = [run_layer(1, o) for o in outs]
                outs = [run_layer(2, o, out_dtype=F32) for o in outs]
                for s, o2 in zip(sl, outs):
                    # L2 output is a single stacked [128, 1024] tile
                    nc.sync.dma_start(
                        out=outT[:, s * 1024 : (s + 1) * 1024], in_=o2.tail[:]
                    )

    return nc


_NC_CACHE = {}


def _get_nc(repeat=1):
    if repeat not in _NC_CACHE:
        _NC_CACHE[repeat] = _split_multi_waits(_build_nc(repeat))
    return _NC_CACHE[repeat]


# ------------------------------------------------------------------ host
def _prep_host_inputs(inputs):
    """Fold masks / t-diff, pack biases, shard x.  Returns per-core maps."""
    f32 = np.float32
    common = {}
    folded = {}
    for l, (ind, hid) in enumerate(LAYER_DIMS):
        m = inputs[f"mask_{l}"][:ind].astype(f32)
        folded[(l, "f1")] = (inputs[f"Wf1_{l}"][:ind] * m).astype(f32)
        folded[(l, "f2")] = (inputs[f"Wf2_{l}"][:ind] * m).astype(f32)
        folded[(l, "t")] = (
            inputs[f"Wtb_{l}"][:ind] - inputs[f"Wta_{l}"][:ind]
        ).astype(f32)
    for mat in MATS:  # zero-pad L0 contraction rows 74..127
        W = folded[(0, mat)]
        folded[(0, mat)] = np.concatenate(
            [W, np.zeros((K0_PAD - W.shape[0], W.shape[1]), f32)], axis=0
        )
    wpack = np.zeros((128, NW), dtype=f32)
    for (l, mat, mkind, mi, kkind, ki), (c0, msz) in WPACK_COLS.items():
        W = folded[(l, mat)]
        _, hid = LAYER_DIMS[l]
        m0 = mi * 128 if mkind == "full" else (hid // 128) * 128
        rm = min(msz, hid - m0)
        kp = [p for p in _in_kparts(l) if p[0] == kkind and p[1] == ki][0]
        _, _, k0, ksz = kp
        if kkind == "ktail":
            pnf, pr, pst, png, pcpg = _tail_spec(LAYER_DIMS[l - 1][1])
            for g in range(png):
                wpack[pst * g : pst * g + ksz, c0 : c0 + rm] = W[
                    k0 : k0 + ksz, m0 : m0 + rm
                ]
        else:
            wpack[:ksz, c0 : c0 + rm] = W[k0 : k0 + ksz, m0 : m0 + rm]
    import ml_dtypes
    np_mm = mybir.dt.np(MM_DT)
    common["wpack"] = wpack.astype(np_mm)
    biases = np.zeros((128, N_BIAS_COLS), dtype=f32)
    for l, (ind, hid) in enumerate(LAYER_DIMS):
        n_full, r, stride, ngroups, cpg = _tail_spec(hid)
        bmats = {
            "f1": inputs[f"bf1_{l}"],
            "f2": inputs[f"bf2_{l}"],
            "t": inputs[f"btb_{l}"] - inputs[f"bta_{l}"],
        }
        for mat, b in bmats.items():
            for mi in range(n_full):
                biases[:, BIAS_COLS[(l, mat, "full", mi)]] = b[
                    mi * 128 : (mi + 1) * 128
                ]
            if r:
                col = BIAS_COLS[(l, mat, "tail")]
                for g in range(ngroups):
                    biases[g * stride : g * stride + r, col] = b[
                        n_full * 128 : n_full * 128 + r
                    ]
    common["biases"] = biases

    xT = np.zeros((K0_PAD, BATCH), dtype=np_mm)
    xT[:INPUT_DIM] = np.asarray(inputs["x"], dtype=f32).T.astype(np_mm)
    in_maps = []
    for c in range(N_CORES):
        m = dict(common)
        m["xT"] = np.ascontiguousarray(xT[:, c * B_CORE : (c + 1) * B_CORE])
        in_maps.append(m)
    return in_maps


def _unpack_out(outT_core):
    """[128, B_CORE//2] stacked -> [B_CORE, 64].

    Per slice s, column block [:, s*1024:(s+1)*1024]: rows 64g..64g+64
    hold batch chunks (2g, 2g+1) of that slice at free offsets 0/512.
    """
    out = np.empty((B_CORE, MOTOR), dtype=outT_core.dtype)
    for s in range(SLICES):
        blk = outT_core[:, s * 1024 : (s + 1) * 1024]
        for c in range(NCH):
            g, f0 = c // 2, (c % 2) * 512
            rows = slice(s * G + c * 512, s * G + (c + 1) * 512)
            out[rows, :] = blk[64 * g : 64 * g + 64, f0 : f0 + 512].T
    return out


def run(inputs, trace=False, repeat=1, **kw):
    """Run on hardware; returns (out [BATCH, MOTOR] fp32, results)."""
    nc = _get_nc(repeat)
    in_maps = _prep_host_inputs(inputs)
    res = run_bass_kernel_spmd(
        nc, in_maps, core_ids=list(range(N_CORES)), trace=trace, **kw
    )
    out = np.empty((BATCH, MOTOR), dtype=np.float32)
    for c in range(N_CORES):
        out[c * B_CORE : (c + 1) * B_CORE, :] = _unpack_out(res.results[c]["outT"])
    return out, res


def kernel(**inputs) -> np.ndarray:
    out, _ = run(inputs, trace=False)
    return out



# revision 20
# speedup vs baseline: 1.5576x; 1.3336x over previous
"""CfC head (3 stacked CfC cells, seq_len=1, h0=0) on 8 TRN2 NeuronCores.

Math (per cell, zero initial hidden state, ts=1):
    ff1 = tanh(x @ (Wf1*mask)[:in] + bf1)
    ff2 = tanh(x @ (Wf2*mask)[:in] + bf2)
    s   = sigmoid(x @ (Wtb - Wta)[:in] + (btb - bta))
    out = ff1 + s * (ff2 - ff1)

Because h0 == 0 only the first in_dim rows of each weight matter, the
sparsity mask folds into the weights, and t_a/t_b fold into a single
matmul.  That is all O(params) host-side prep; the O(B) work runs on
the NeuronCores, data-parallel over the batch.

Device layout is feature-major ([feat, batch]); x is transposed on the
host so every DMA is contiguous per partition.  Per 2048-column slice,
each layer is computed as a set of "instances":
  - full 128-row M-tiles, processed in two 1024-column halves
    (PSUM tiles are [128,1024] = 2 banks, 4 pool slots -> deep
    PE/ACT overlap), and
  - the M-tail (hid % 128 = 13/51/64 rows), batch-STACKED across
    column-tile positions of the PE array: 4 (or 2) batch chunks are
    packed onto partition groups at 32/64-partition offsets via
    tile_position, so the tail costs one narrow ACT/DVE op instead of
    four wide ones, and its matmuls run concurrently on distinct PE
    column groups.
The next layer contracts over the previous outputs in place: stacked
tail rows are consumed by row-positioned (tile_position) K-tail
matmuls against weight tiles replicated at the matching partition
offsets.
"""

import numpy as np

import concourse.bass as bass
import concourse.tile as tile
from concourse import mybir
from concourse.bass_utils import run_bass_kernel_spmd

# ---------------------------------------------------------------- dims
INPUT_DIM, INTER, COMMAND, MOTOR = 74, 269, 179, 64
# L0's contraction is zero-padded 74 -> 128: the PE activity monitor
# only counts full-row (K=128) matmuls, so K=74 work runs at the cold
# 1.2 GHz clock forever.  Full-K L0 matmuls keep the PE at 2.4 GHz.
K0_PAD = 128
BATCH = 65536
N_CORES = 8
B_CORE = BATCH // N_CORES          # 8192 rows per core
G = 2048                           # batch columns per pipeline slice
NCH = G // 512                     # 512-column matmul chunks per slice
SLICES = B_CORE // G

LAYER_DIMS = [(INPUT_DIM, INTER), (INTER, COMMAND), (COMMAND, MOTOR)]
MATS = ("f1", "f2", "t")
F32 = mybir.dt.float32
F32R = mybir.dt.float32r
# matmul operand / activation storage dtype.  bf16 allows PE column
# tile_position (stacked tails) and fast weight load; PSUM accumulation
# stays fp32.  (f32r rejects column tile positions in this walrus
# build.)
MM_DT = mybir.dt.bfloat16
# lerp elementwise dtype: fp16 keeps 10 mantissa bits and runs the DVE
# tensor_tensor ops in the 2x 16-bit mode.
EW_DT = mybir.dt.float16


def _tail_spec(hid):
    """(n_full, r, stride, ngroups, cpg) for a layer's M dimension."""
    n_full = hid // 128
    r = hid - 128 * n_full
    if r == 0:
        return n_full, 0, 0, 0, 0
    stride = 32 if r <= 32 else 64
    ngroups = 128 // stride
    cpg = NCH // ngroups           # batch chunks stacked per partition group
    return n_full, r, stride, ngroups, cpg


def _instances(hid):
    # tail first: its output gates the next layer's K-tail matmuls, so
    # let its (cheap) elementwise chain run under the fulls' matmuls.
    # Each full instance covers the whole slice (all NCH chunks) in one
    # 4-bank PSUM tile -> one wide ACT per mat.
    n_full, r, stride, ngroups, cpg = _tail_spec(hid)
    out = []
    if r:
        out.append(("tail",))
    for mi in range(n_full):
        out.append(("full", mi))
    return out


# bias pack columns: one per (layer, mat, m-range); full halves share.
BIAS_COLS = {}
_c = 0
for _l, (_in, _hid) in enumerate(LAYER_DIMS):
    _nf, _r, _st, _ng, _cpg = _tail_spec(_hid)
    for _mat in MATS:
        for _mi in range(_nf):
            BIAS_COLS[(_l, _mat, "full", _mi)] = _c
            _c += 1
        if _r:
            BIAS_COLS[(_l, _mat, "tail")] = _c
            _c += 1
N_BIAS_COLS = _c


def _mranges(l):
    # output m-ranges; tail m-size padded to the group stride with zero
    # columns so stacked PSUM groups cover every partition
    nf, r, st, ng, cpg = _tail_spec(LAYER_DIMS[l][1])
    out = [("full", mi, mi * 128, 128) for mi in range(nf)]
    if r:
        out.append(("tail", None, nf * 128, st))
    return out


def _in_kparts(l):
    if l == 0:
        return [("main", 0, 0, K0_PAD)]
    nf, r, st, ng, cpg = _tail_spec(LAYER_DIMS[l - 1][1])
    parts = [("main", ki, ki * 128, 128) for ki in range(nf)]
    if r:
        parts.append(("ktail", None, nf * 128, r))
    return parts


def _wpack_layout():
    # all lhsT tiles as column blocks of one [128, NW] array; k-tail
    # blocks carry weight rows replicated at each stacked group base
    cols = {}
    c = 0
    for l in range(len(LAYER_DIMS)):
        for mat in MATS:
            for mkind, mi, m0, msz in _mranges(l):
                for kkind, ki, k0, ksz in _in_kparts(l):
                    cols[(l, mat, mkind, mi, kkind, ki)] = (c, msz)
                    c += msz
    return cols, c


WPACK_COLS, NW = _wpack_layout()


# ---------------------------------------------- walrus sync-wait workaround
def _split_multi_waits(nc):
    """This walrus build accepts only ONE sync-wait command per
    instruction.  Tile attaches one wait per outstanding proc, so after
    scheduling, hoist every excess wait onto a single-wait NOP emitted
    just before the instruction on the same engine (engine queues are
    in-order, so the waits still all complete before it executes)."""
    import bass_rust as _br

    for fn in nc.m.functions:
        for blk in fn.blocks:
            out = []
            changed = False
            for inst in blk.instructions:
                si = inst.sync_info
                if si is not None and len(si.on_wait) > 1:
                    waits = list(si.on_wait)
                    for j, w in enumerate(waits[:-1]):
                        carrier = mybir.InstNoOp(
                            name=f"{inst.name}-sw{j}", engine=inst.engine
                        )
                        carrier.sync_info = _br.SyncInfo(on_wait=[w], on_update=[])
                        out.append(carrier)
                    inst.sync_info = _br.SyncInfo(
                        on_wait=[waits[-1]], on_update=list(si.on_update)
                    )
                    changed = True
                out.append(inst)
            if changed:
                blk.instructions = out
    return nc


# ---------------------------------------------------------------- device
class _LayerOut:
    """Feature-major activation of one layer for one slice.

    fulls: mi -> [128, 512*NCH] tile (feature rows mi*128..+128, all
           batch chunks of the slice).
    tail:  [128, 512*cpg] tile; partition group g holds feature rows
           n_full*128..+r for batch chunks g*cpg..(g+1)*cpg.
    """

    def __init__(self, hid):
        self.hid = hid
        self.n_full, self.r, self.stride, self.ngroups, self.cpg = _tail_spec(hid)
        self.fulls = {}
        self.tail = None

    def kparts(self):
        parts = [("main", ki, ki * 128, 128) for ki in range(self.n_full)]
        if self.r:
            parts.append(("ktail", None, self.n_full * 128, self.r))
        return parts

    def rhs(self, kind, ki, c):
        """(ap, row_pos) of this output as contraction input, chunk c."""
        if kind == "main":
            t = self.fulls[ki]
            return t[:, c * 512 : (c + 1) * 512], 0
        g = c // self.cpg
        p0 = self.stride * g
        f0 = (c % self.cpg) * 512
        return self.tail[p0 : p0 + self.r, f0 : f0 + 512], p0


def _build_nc(repeat=1):
    nc = bass.Bass(target_bir_lowering=False)

    xT = nc.dram_tensor("xT", [K0_PAD, B_CORE], MM_DT, kind="ExternalInput")
    wpack_dram = nc.dram_tensor("wpack", [128, NW], MM_DT, kind="ExternalInput")
    bias_dram = nc.dram_tensor("biases", [128, N_BIAS_COLS], F32, kind="ExternalInput")
    # stacked output layout; host unpacks (see _unpack_out)
    outT = nc.dram_tensor("outT", [128, B_CORE // 2], F32, kind="ExternalOutput")

    TANH = mybir.ActivationFunctionType.Tanh
    SIGM = mybir.ActivationFunctionType.Sigmoid

    with tile.TileContext(nc) as tc:
        with (
            tc.tile_pool(name="consts", bufs=1) as consts,
            tc.tile_pool(name="xt", bufs=3) as xt_pool,
            tc.tile_pool(name="act", bufs=2) as act_pool,
            tc.tile_pool(name="ff", bufs=3) as ff_pool,
            # two 4-bank [128, 2048] psum tiles: PE fills one while ACT
            # drains the other; wide tiles quarter the ACT instruction
            # count and shrink layer-boundary fill bubbles
            tc.tile_pool(name="ps", bufs=2, space="PSUM") as ps_pool,
        ):
            # consts arrive via the ACT HWDGE ring so the SP ring can
            # start streaming x immediately
            bias_sb = consts.tile([128, N_BIAS_COLS], F32, tag="bias")
            nc.scalar.dma_start(out=bias_sb[:], in_=bias_dram[:])
            wpack_sb = consts.tile([128, NW], MM_DT, tag="wpack")
            nc.scalar.dma_start(out=wpack_sb[:], in_=wpack_dram[:])

            # PE warm-up: the HAM activity monitor only registers
            # full-row (K=128) matmul activity, so a kernel that opens
            # with K=74 L0 matmuls runs its entire span at the cold
            # 1.2 GHz clock.  A short block of full-K zero matmuls
            # (overlapping the initial DMA wait) trips the un-throttle;
            # the real work then issues at 2.4 GHz, and partial-K
            # activity keeps the PE from re-throttling.
            # ~8 MMs (3.4us) trip the un-throttle; the rest keep the PE
            # busy across the wpack/xt DMA wait (~10us) so the HAM MID
            # window (~5.2us idle) can't re-throttle before real work.
            N_WARM = 32
            warm = consts.tile([128, 640], MM_DT, tag="warm")
            nc.vector.memset(warm[:], 0.0)
            warm_ps = ps_pool.tile([128, 512], F32, tag="ps")
            for i in range(N_WARM):
                nc.tensor.matmul(
                    warm_ps[:],
                    warm[:, 0:128],
                    warm[:, 128:640],
                    start=(i == 0),
                    stop=(i == N_WARM - 1),
                )

            # ---- per-slice pipeline (repeat>1 reruns the same work for
            # differential wall-clock timing; outputs just overwritten)
            def make_xt_in(s):
                c0 = s * G
                xt = xt_pool.tile([K0_PAD, G], MM_DT, tag="xt")
                nc.sync.dma_start(out=xt[:], in_=xT[:, c0 : c0 + G])

                class _XtIn:
                    @staticmethod
                    def kparts():
                        return [("main", 0, 0, K0_PAD)]

                    @staticmethod
                    def rhs(kind, ki, c):
                        return xt[:, c * 512 : (c + 1) * 512], 0

                return _XtIn

            if True:

                def layer_tasks(l, lin, out_dtype=MM_DT):
                    """Return (lo, thunks): one thunk per instance;
                    running a thunk emits that instance's IR and files
                    its output tile into lo."""
                    lo = _LayerOut(LAYER_DIMS[l][1])
                    kps = lin.kparts()

                    def instance(inst):
                        if inst[0] == "full":
                            _, mi = inst
                            P, FF = 128, 512 * NCH
                            chunks = list(range(NCH))
                            mkind, mmi = "full", mi

                            def region(c):
                                f0 = c * 512
                                return slice(0, 128), slice(f0, f0 + 512), 0
                        else:
                            P = 128
                            FF = 512 * lo.cpg
                            # alternate column groups so adjacent matmuls
                            # land on distinct PE column tiles
                            chunks = sorted(
                                range(NCH), key=lambda c: (c % lo.cpg, c)
                            )
                            mkind, mmi = "tail", None

                            def region(c):
                                g = c // lo.cpg
                                p0 = lo.stride * g
                                f0 = (c % lo.cpg) * 512
                                return (
                                    slice(p0, p0 + lo.stride),
                                    slice(f0, f0 + 512),
                                    p0,
                                )

                        ff = {}
                        for mat in MATS:
                            ps = ps_pool.tile([P, FF], F32, tag="ps")
                            # kpart-major: one stationary-weight load
                            # serves all chunks; each chunk's bank still
                            # sees its kpi==0 matmul first
                            for kpi, (kkind, ki, k0, ksz) in enumerate(kps):
                                wc0, wmsz = WPACK_COLS[
                                    (l, mat, mkind, mmi, kkind, ki)
                                ]
                                for c in chunks:
                                    psl, fsl, colp = region(c)
                                    rhs_ap, rowp = lin.rhs(kkind, ki, c)
                                    lhsT = wpack_sb[
                                        rowp : rowp + ksz, wc0 : wc0 + wmsz
                                    ]
                                    nc.tensor.matmul(
                                        ps[psl, fsl],
                                        lhsT,
                                        rhs_ap,
                                        start=(kpi == 0),
                                        stop=(kpi == len(kps) - 1),
                                        tile_position=(rowp, colp),
                                    )
                            f = ff_pool.tile([P, FF], EW_DT, tag=f"ff_{mat}")
                            bcol = BIAS_COLS[
                                (l, mat, "full", inst[1])
                                if inst[0] == "full"
                                else (l, mat, "tail")
                            ]
                            nc.scalar.activation(
                                out=f[:],
                                in_=ps[:],
                                func=SIGM if mat == "t" else TANH,
                                bias=bias_sb[:P, bcol : bcol + 1],
                            )
                            ff[mat] = f
                        # out = ff1 + s*(ff2-ff1); 16-bit DVE chain
                        d = ff_pool.tile([P, FF], EW_DT, tag="d")
                        nc.vector.tensor_sub(d[:], ff["f2"][:], ff["f1"][:])
                        nc.vector.tensor_mul(ff["f2"][:], ff["t"][:], d[:])
                        tag = (
                            f"o{l}_{inst[1]}"
                            if inst[0] == "full"
                            else f"o{l}_tail"
                        )
                        o = act_pool.tile([P, FF], out_dtype, tag=tag)
                        nc.vector.tensor_add(o[:], ff["f1"][:], ff["f2"][:])
                        return o

                    def run_inst(inst):
                        o = instance(inst)
                        if inst[0] == "full":
                            lo.fulls[inst[1]] = o
                        else:
                            lo.tail = o

                    thunks = [
                        (lambda inst=inst: run_inst(inst))
                        for inst in _instances(lo.hid)
                    ]
                    return lo, thunks

            # process slices in pairs, layer-major: dense same-layer
            # matmul streams keep the PE activity monitor warm (mixing
            # layers dilutes the full-K density and re-throttles the
            # PE); each layer's fill-latency overlaps the sibling
            # slice's dense work
            PAIR = 2
            total = SLICES * repeat

            def run_layer(l, lin, out_dtype=MM_DT):
                lo, thunks = layer_tasks(l, lin, out_dtype)
                for th in thunks:
                    th()
                return lo

            for pr in range(0, total, PAIR):
                sl = [(pr + j) % SLICES for j in range(min(PAIR, total - pr))]
                outs = [make_xt_in(s) for s in sl]
                outs = [run_layer(0, x) for x in outs]
                outs = [run_layer(1, o) for o in outs]
                outs = [run_layer(2, o, out_dtype=F32) for o in outs]
                for s, o2 in zip(sl, outs):
                    # L2 output is a single stacked [128, 1024] tile
                    nc.sync.dma_start(
                        out=outT[:, s * 1024 : (s + 1) * 1024], in_=o2.tail[:]
                    )

    return nc


_NC_CACHE = {}


def _get_nc(repeat=1):
    if repeat not in _NC_CACHE:
        _NC_CACHE[repeat] = _split_multi_waits(_build_nc(repeat))
    return _NC_CACHE[repeat]


# ------------------------------------------------------------------ host
def _prep_host_inputs(inputs):
    """Fold masks / t-diff, pack biases, shard x.  Returns per-core maps."""
    f32 = np.float32
    common = {}
    folded = {}
    for l, (ind, hid) in enumerate(LAYER_DIMS):
        m = inputs[f"mask_{l}"][:ind].astype(f32)
        folded[(l, "f1")] = (inputs[f"Wf1_{l}"][:ind] * m).astype(f32)
        folded[(l, "f2")] = (inputs[f"Wf2_{l}"][:ind] * m).astype(f32)
        folded[(l, "t")] = (
            inputs[f"Wtb_{l}"][:ind] - inputs[f"Wta_{l}"][:ind]
        ).astype(f32)
    for mat in MATS:  # zero-pad L0 contraction rows 74..127
        W = folded[(0, mat)]
        folded[(0, mat)] = np.concatenate(
            [W, np.zeros((K0_PAD - W.shape[0], W.shape[1]), f32)], axis=0
        )
    wpack = np.zeros((128, NW), dtype=f32)
    for (l, mat, mkind, mi, kkind, ki), (c0, msz) in WPACK_COLS.items():
        W = folded[(l, mat)]
        _, hid = LAYER_DIMS[l]
        m0 = mi * 128 if mkind == "full" else (hid // 128) * 128
        rm = min(msz, hid - m0)
        kp = [p for p in _in_kparts(l) if p[0] == kkind and p[1] == ki][0]
        _, _, k0, ksz = kp
        if kkind == "ktail":
            pnf, pr, pst, png, pcpg = _tail_spec(LAYER_DIMS[l - 1][1])
            for g in range(png):
                wpack[pst * g : pst * g + ksz, c0 : c0 + rm] = W[
                    k0 : k0 + ksz, m0 : m0 + rm
                ]
        else:
            wpack[:ksz, c0 : c0 + rm] = W[k0 : k0 + ksz, m0 : m0 + rm]
    import ml_dtypes
    np_mm = mybir.dt.np(MM_DT)
    common["wpack"] = wpack.astype(np_mm)
    biases = np.zeros((128, N_BIAS_COLS), dtype=f32)
    for l, (ind, hid) in enumerate(LAYER_DIMS):
        n_full, r, stride, ngroups, cpg = _tail_spec(hid)
        bmats = {
            "f1": inputs[f"bf1_{l}"],
            "f2": inputs[f"bf2_{l}"],
            "t": inputs[f"btb_{l}"] - inputs[f"bta_{l}"],
        }
        for mat, b in bmats.items():
            for mi in range(n_full):
                biases[:, BIAS_COLS[(l, mat, "full", mi)]] = b[
                    mi * 128 : (mi + 1) * 128
                ]
            if r:
                col = BIAS_COLS[(l, mat, "tail")]
                for g in range(ngroups):
                    biases[g * stride : g * stride + r, col] = b[
                        n_full * 128 : n_full * 128 + r
                    ]
    common["biases"] = biases

    xT = np.zeros((K0_PAD, BATCH), dtype=np_mm)
    xT[:INPUT_DIM] = np.asarray(inputs["x"], dtype=f32).T.astype(np_mm)
    in_maps = []
    for c in range(N_CORES):
        m = dict(common)
        m["xT"] = np.ascontiguousarray(xT[:, c * B_CORE : (c + 1) * B_CORE])
        in_maps.append(m)
    return in_maps


def _unpack_out(outT_core):
    """[128, B_CORE//2] stacked -> [B_CORE, 64].

    Per slice s, column block [:, s*1024:(s+1)*1024]: rows 64g..64g+64
    hold batch chunks (2g, 2g+1) of that slice at free offsets 0/512.
    """
    out = np.empty((B_CORE, MOTOR), dtype=outT_core.dtype)
    for s in range(SLICES):
        blk = outT_core[:, s * 1024 : (s + 1) * 1024]
        for c in range(NCH):
            g, f0 = c // 2, (c % 2) * 512
            rows = slice(s * G + c * 512, s * G + (c + 1) * 512)
            out[rows, :] = blk[64 * g : 64 * g + 64, f0 : f0 + 512].T
    return out


def run(inputs, trace=False, repeat=1, **kw):
    """Run on hardware; returns (out [BATCH, MOTOR] fp32, results)."""
    nc = _get_nc(repeat)
    in_maps = _prep_host_inputs(inputs)
    res = run_bass_kernel_spmd(
        nc, in_maps, core_ids=list(range(N_CORES)), trace=trace, **kw
    )
    out = np.empty((BATCH, MOTOR), dtype=np.float32)
    for c in range(N_CORES):
        out[c * B_CORE : (c + 1) * B_CORE, :] = _unpack_out(res.results[c]["outT"])
    return out, res


def kernel(**inputs) -> np.ndarray:
    out, _ = run(inputs, trace=False)
    return out



# revision 23
# speedup vs baseline: 1.5670x; 1.0060x over previous
"""CfC head (3 stacked CfC cells, seq_len=1, h0=0) on 8 TRN2 NeuronCores.

Math (per cell, zero initial hidden state, ts=1):
    ff1 = tanh(x @ (Wf1*mask)[:in] + bf1)
    ff2 = tanh(x @ (Wf2*mask)[:in] + bf2)
    s   = sigmoid(x @ (Wtb - Wta)[:in] + (btb - bta))
    out = ff1 + s * (ff2 - ff1)

Because h0 == 0 only the first in_dim rows of each weight matter, the
sparsity mask folds into the weights, and t_a/t_b fold into a single
matmul.  That is all O(params) host-side prep; the O(B) work runs on
the NeuronCores, data-parallel over the batch.

Device layout is feature-major ([feat, batch]); x is transposed on the
host so every DMA is contiguous per partition.  Per 2048-column slice,
each layer is computed as a set of "instances":
  - full 128-row M-tiles, processed in two 1024-column halves
    (PSUM tiles are [128,1024] = 2 banks, 4 pool slots -> deep
    PE/ACT overlap), and
  - the M-tail (hid % 128 = 13/51/64 rows), batch-STACKED across
    column-tile positions of the PE array: 4 (or 2) batch chunks are
    packed onto partition groups at 32/64-partition offsets via
    tile_position, so the tail costs one narrow ACT/DVE op instead of
    four wide ones, and its matmuls run concurrently on distinct PE
    column groups.
The next layer contracts over the previous outputs in place: stacked
tail rows are consumed by row-positioned (tile_position) K-tail
matmuls against weight tiles replicated at the matching partition
offsets.
"""

import numpy as np

import concourse.bass as bass
import concourse.tile as tile
from concourse import mybir
from concourse.bass_utils import run_bass_kernel_spmd

# ---------------------------------------------------------------- dims
INPUT_DIM, INTER, COMMAND, MOTOR = 74, 269, 179, 64
# L0's contraction is zero-padded 74 -> 128: the PE activity monitor
# only counts full-row (K=128) matmuls, so K=74 work runs at the cold
# 1.2 GHz clock forever.  Full-K L0 matmuls keep the PE at 2.4 GHz.
K0_PAD = 128
BATCH = 65536
N_CORES = 8
B_CORE = BATCH // N_CORES          # 8192 rows per core
G = 2048                           # batch columns per pipeline slice
NCH = G // 512                     # 512-column matmul chunks per slice
SLICES = B_CORE // G

LAYER_DIMS = [(INPUT_DIM, INTER), (INTER, COMMAND), (COMMAND, MOTOR)]
MATS = ("f1", "f2", "t")
F32 = mybir.dt.float32
F32R = mybir.dt.float32r
# matmul operand / activation storage dtype.  bf16 allows PE column
# tile_position (stacked tails) and fast weight load; PSUM accumulation
# stays fp32.  (f32r rejects column tile positions in this walrus
# build.)
MM_DT = mybir.dt.bfloat16
# lerp elementwise dtype: fp16 keeps 10 mantissa bits and runs the DVE
# tensor_tensor ops in the 2x 16-bit mode.
EW_DT = mybir.dt.float16


def _tail_spec(hid):
    """(n_full, r, stride, ngroups, cpg) for a layer's M dimension."""
    n_full = hid // 128
    r = hid - 128 * n_full
    if r == 0:
        return n_full, 0, 0, 0, 0
    stride = 32 if r <= 32 else 64
    ngroups = 128 // stride
    cpg = NCH // ngroups           # batch chunks stacked per partition group
    return n_full, r, stride, ngroups, cpg


def _instances(hid):
    # tail first: its output gates the next layer's K-tail matmuls, so
    # let its (cheap) elementwise chain run under the fulls' matmuls.
    # Each full instance covers the whole slice (all NCH chunks) in one
    # 4-bank PSUM tile -> one wide ACT per mat.
    n_full, r, stride, ngroups, cpg = _tail_spec(hid)
    out = []
    if r:
        out.append(("tail",))
    for mi in range(n_full):
        out.append(("full", mi))
    return out


# bias pack columns: one per (layer, mat, m-range); full halves share.
BIAS_COLS = {}
_c = 0
for _l, (_in, _hid) in enumerate(LAYER_DIMS):
    _nf, _r, _st, _ng, _cpg = _tail_spec(_hid)
    for _mat in MATS:
        for _mi in range(_nf):
            BIAS_COLS[(_l, _mat, "full", _mi)] = _c
            _c += 1
        if _r:
            BIAS_COLS[(_l, _mat, "tail")] = _c
            _c += 1
N_BIAS_COLS = _c


def _mranges(l):
    # output m-ranges; tail m-size padded to the group stride with zero
    # columns so stacked PSUM groups cover every partition
    nf, r, st, ng, cpg = _tail_spec(LAYER_DIMS[l][1])
    out = [("full", mi, mi * 128, 128) for mi in range(nf)]
    if r:
        out.append(("tail", None, nf * 128, st))
    return out


def _in_kparts(l):
    if l == 0:
        return [("main", 0, 0, K0_PAD)]
    nf, r, st, ng, cpg = _tail_spec(LAYER_DIMS[l - 1][1])
    parts = [("main", ki, ki * 128, 128) for ki in range(nf)]
    if r:
        parts.append(("ktail", None, nf * 128, r))
    return parts


def _wpack_layout():
    # all lhsT tiles as column blocks of one [128, NW] array; k-tail
    # blocks carry weight rows replicated at each stacked group base
    cols = {}
    c = 0
    for l in range(len(LAYER_DIMS)):
        for mat in MATS:
            for mkind, mi, m0, msz in _mranges(l):
                for kkind, ki, k0, ksz in _in_kparts(l):
                    cols[(l, mat, mkind, mi, kkind, ki)] = (c, msz)
                    c += msz
    return cols, c


WPACK_COLS, NW = _wpack_layout()


# ---------------------------------------------- walrus sync-wait workaround
def _split_multi_waits(nc):
    """This walrus build accepts only ONE sync-wait command per
    instruction.  Tile attaches one wait per outstanding proc, so after
    scheduling, hoist every excess wait onto a single-wait NOP emitted
    just before the instruction on the same engine (engine queues are
    in-order, so the waits still all complete before it executes)."""
    import bass_rust as _br

    for fn in nc.m.functions:
        for blk in fn.blocks:
            out = []
            changed = False
            for inst in blk.instructions:
                si = inst.sync_info
                if si is not None and len(si.on_wait) > 1:
                    waits = list(si.on_wait)
                    for j, w in enumerate(waits[:-1]):
                        carrier = mybir.InstNoOp(
                            name=f"{inst.name}-sw{j}", engine=inst.engine
                        )
                        carrier.sync_info = _br.SyncInfo(on_wait=[w], on_update=[])
                        out.append(carrier)
                    inst.sync_info = _br.SyncInfo(
                        on_wait=[waits[-1]], on_update=list(si.on_update)
                    )
                    changed = True
                out.append(inst)
            if changed:
                blk.instructions = out
    return nc


# ---------------------------------------------------------------- device
class _LayerOut:
    """Feature-major activation of one layer for one slice.

    fulls: mi -> [128, 512*NCH] tile (feature rows mi*128..+128, all
           batch chunks of the slice).
    tail:  [128, 512*cpg] tile; partition group g holds feature rows
           n_full*128..+r for batch chunks g*cpg..(g+1)*cpg.
    """

    def __init__(self, hid):
        self.hid = hid
        self.n_full, self.r, self.stride, self.ngroups, self.cpg = _tail_spec(hid)
        self.fulls = {}
        self.tail = None

    def kparts(self):
        parts = [("main", ki, ki * 128, 128) for ki in range(self.n_full)]
        if self.r:
            parts.append(("ktail", None, self.n_full * 128, self.r))
        return parts

    def rhs(self, kind, ki, c):
        """(ap, row_pos) of this output as contraction input, chunk c."""
        if kind == "main":
            t = self.fulls[ki]
            return t[:, c * 512 : (c + 1) * 512], 0
        g = c // self.cpg
        p0 = self.stride * g
        f0 = (c % self.cpg) * 512
        return self.tail[p0 : p0 + self.r, f0 : f0 + 512], p0


def _build_nc(repeat=1):
    nc = bass.Bass(target_bir_lowering=False)

    xT = nc.dram_tensor("xT", [K0_PAD, B_CORE], MM_DT, kind="ExternalInput")
    wpack_dram = nc.dram_tensor("wpack", [128, NW], MM_DT, kind="ExternalInput")
    bias_dram = nc.dram_tensor("biases", [128, N_BIAS_COLS], F32, kind="ExternalInput")
    # stacked output layout; host unpacks (see _unpack_out)
    outT = nc.dram_tensor("outT", [128, B_CORE // 2], F32, kind="ExternalOutput")

    TANH = mybir.ActivationFunctionType.Tanh
    SIGM = mybir.ActivationFunctionType.Sigmoid

    with tile.TileContext(nc) as tc:
        with (
            tc.tile_pool(name="consts", bufs=1) as consts,
            tc.tile_pool(name="xt", bufs=3) as xt_pool,
            tc.tile_pool(name="act", bufs=2) as act_pool,
            tc.tile_pool(name="ff", bufs=4) as ff_pool,
            # two 4-bank [128, 2048] psum tiles: PE fills one while ACT
            # drains the other; wide tiles quarter the ACT instruction
            # count and shrink layer-boundary fill bubbles
            tc.tile_pool(name="ps", bufs=2, space="PSUM") as ps_pool,
        ):
            # consts arrive via the ACT HWDGE ring so the SP ring can
            # start streaming x immediately
            bias_sb = consts.tile([128, N_BIAS_COLS], F32, tag="bias")
            nc.scalar.dma_start(out=bias_sb[:], in_=bias_dram[:])
            wpack_sb = consts.tile([128, NW], MM_DT, tag="wpack")
            nc.scalar.dma_start(out=wpack_sb[:], in_=wpack_dram[:])

            # PE warm-up: the HAM activity monitor only registers
            # full-row (K=128) matmul activity, so a kernel that opens
            # with K=74 L0 matmuls runs its entire span at the cold
            # 1.2 GHz clock.  A short block of full-K zero matmuls
            # (overlapping the initial DMA wait) trips the un-throttle;
            # the real work then issues at 2.4 GHz, and partial-K
            # activity keeps the PE from re-throttling.
            # No explicit PE warm-up block: with L0 padded to K=128 the
            # first real matmuls trip the HAM un-throttle themselves
            # (~8 cold MMs, one-time ~2.5us) — cheaper than queueing a
            # dummy block ahead of them.
            # Dummy activation: forces the ~1.3us ACT table load to run
            # during the DMA wait instead of before the first real
            # activation (tile never read).
            act_warm = consts.tile([128, 8], F32, tag="actwarm")
            nc.scalar.activation(
                out=act_warm[:], in_=bias_sb[:, 0:8], func=TANH
            )

            # ---- per-slice pipeline (repeat>1 reruns the same work for
            # differential wall-clock timing; outputs just overwritten)
            def make_xt_in(s):
                c0 = s * G
                xt = xt_pool.tile([K0_PAD, G], MM_DT, tag="xt")
                nc.sync.dma_start(out=xt[:], in_=xT[:, c0 : c0 + G])

                class _XtIn:
                    @staticmethod
                    def kparts():
                        return [("main", 0, 0, K0_PAD)]

                    @staticmethod
                    def rhs(kind, ki, c):
                        return xt[:, c * 512 : (c + 1) * 512], 0

                return _XtIn

            if True:

                def layer_tasks(l, lin, out_dtype=MM_DT):
                    """Return (lo, thunks): one thunk per instance;
                    running a thunk emits that instance's IR and files
                    its output tile into lo."""
                    lo = _LayerOut(LAYER_DIMS[l][1])
                    kps = lin.kparts()

                    def instance(inst):
                        if inst[0] == "full":
                            _, mi = inst
                            P, FF = 128, 512 * NCH
                            chunks = list(range(NCH))
                            mkind, mmi = "full", mi

                            def region(c):
                                f0 = c * 512
                                return slice(0, 128), slice(f0, f0 + 512), 0
                        else:
                            P = 128
                            FF = 512 * lo.cpg
                            # alternate column groups so adjacent matmuls
                            # land on distinct PE column tiles
                            chunks = sorted(
                                range(NCH), key=lambda c: (c % lo.cpg, c)
                            )
                            mkind, mmi = "tail", None

                            def region(c):
                                g = c // lo.cpg
                                p0 = lo.stride * g
                                f0 = (c % lo.cpg) * 512
                                return (
                                    slice(p0, p0 + lo.stride),
                                    slice(f0, f0 + 512),
                                    p0,
                                )

                        ff = {}
                        for mat in MATS:
                            ps = ps_pool.tile([P, FF], F32, tag="ps")
                            # kpart-major: one stationary-weight load
                            # serves all chunks; each chunk's bank still
                            # sees its kpi==0 matmul first
                            for kpi, (kkind, ki, k0, ksz) in enumerate(kps):
                                wc0, wmsz = WPACK_COLS[
                                    (l, mat, mkind, mmi, kkind, ki)
                                ]
                                for c in chunks:
                                    psl, fsl, colp = region(c)
                                    rhs_ap, rowp = lin.rhs(kkind, ki, c)
                                    lhsT = wpack_sb[
                                        rowp : rowp + ksz, wc0 : wc0 + wmsz
                                    ]
                                    nc.tensor.matmul(
                                        ps[psl, fsl],
                                        lhsT,
                                        rhs_ap,
                                        start=(kpi == 0),
                                        stop=(kpi == len(kps) - 1),
                                        tile_position=(rowp, colp),
                                    )
                            f = ff_pool.tile([P, FF], EW_DT, tag=f"ff_{mat}")
                            bcol = BIAS_COLS[
                                (l, mat, "full", inst[1])
                                if inst[0] == "full"
                                else (l, mat, "tail")
                            ]
                            nc.scalar.activation(
                                out=f[:],
                                in_=ps[:],
                                func=SIGM if mat == "t" else TANH,
                                bias=bias_sb[:P, bcol : bcol + 1],
                            )
                            ff[mat] = f
                        # out = ff1 + s*(ff2-ff1); 16-bit DVE chain
                        d = ff_pool.tile([P, FF], EW_DT, tag="d")
                        nc.vector.tensor_sub(d[:], ff["f2"][:], ff["f1"][:])
                        nc.vector.tensor_mul(ff["f2"][:], ff["t"][:], d[:])
                        tag = (
                            f"o{l}_{inst[1]}"
                            if inst[0] == "full"
                            else f"o{l}_tail"
                        )
                        o = act_pool.tile([P, FF], out_dtype, tag=tag)
                        nc.vector.tensor_add(o[:], ff["f1"][:], ff["f2"][:])
                        return o

                    def run_inst(inst):
                        o = instance(inst)
                        if inst[0] == "full":
                            lo.fulls[inst[1]] = o
                        else:
                            lo.tail = o

                    thunks = [
                        (lambda inst=inst: run_inst(inst))
                        for inst in _instances(lo.hid)
                    ]
                    return lo, thunks

            # process slices in pairs, layer-major: dense same-layer
            # matmul streams keep the PE activity monitor warm (mixing
            # layers dilutes the full-K density and re-throttles the
            # PE); each layer's fill-latency overlaps the sibling
            # slice's dense work
            PAIR = 2
            total = SLICES * repeat

            def run_layer(l, lin, out_dtype=MM_DT):
                lo, thunks = layer_tasks(l, lin, out_dtype)
                for th in thunks:
                    th()
                return lo

            for pr in range(0, total, PAIR):
                sl = [(pr + j) % SLICES for j in range(min(PAIR, total - pr))]
                outs = [make_xt_in(s) for s in sl]
                outs = [run_layer(0, x) for x in outs]
                outs = [run_layer(1, o) for o in outs]
                outs = [run_layer(2, o, out_dtype=F32) for o in outs]
                for s, o2 in zip(sl, outs):
                    # L2 output is a single stacked [128, 1024] tile
                    nc.sync.dma_start(
                        out=outT[:, s * 1024 : (s + 1) * 1024], in_=o2.tail[:]
                    )

    return nc


_NC_CACHE = {}


def _get_nc(repeat=1):
    if repeat not in _NC_CACHE:
        _NC_CACHE[repeat] = _split_multi_waits(_build_nc(repeat))
    return _NC_CACHE[repeat]


# ------------------------------------------------------------------ host
def _prep_host_inputs(inputs):
    """Fold masks / t-diff, pack biases, shard x.  Returns per-core maps."""
    f32 = np.float32
    common = {}
    folded = {}
    for l, (ind, hid) in enumerate(LAYER_DIMS):
        m = inputs[f"mask_{l}"][:ind].astype(f32)
        folded[(l, "f1")] = (inputs[f"Wf1_{l}"][:ind] * m).astype(f32)
        folded[(l, "f2")] = (inputs[f"Wf2_{l}"][:ind] * m).astype(f32)
        folded[(l, "t")] = (
            inputs[f"Wtb_{l}"][:ind] - inputs[f"Wta_{l}"][:ind]
        ).astype(f32)
    for mat in MATS:  # zero-pad L0 contraction rows 74..127
        W = folded[(0, mat)]
        folded[(0, mat)] = np.concatenate(
            [W, np.zeros((K0_PAD - W.shape[0], W.shape[1]), f32)], axis=0
        )
    wpack = np.zeros((128, NW), dtype=f32)
    for (l, mat, mkind, mi, kkind, ki), (c0, msz) in WPACK_COLS.items():
        W = folded[(l, mat)]
        _, hid = LAYER_DIMS[l]
        m0 = mi * 128 if mkind == "full" else (hid // 128) * 128
        rm = min(msz, hid - m0)
        kp = [p for p in _in_kparts(l) if p[0] == kkind and p[1] == ki][0]
        _, _, k0, ksz = kp
        if kkind == "ktail":
            pnf, pr, pst, png, pcpg = _tail_spec(LAYER_DIMS[l - 1][1])
            for g in range(png):
                wpack[pst * g : pst * g + ksz, c0 : c0 + rm] = W[
                    k0 : k0 + ksz, m0 : m0 + rm
                ]
        else:
            wpack[:ksz, c0 : c0 + rm] = W[k0 : k0 + ksz, m0 : m0 + rm]
    import ml_dtypes
    np_mm = mybir.dt.np(MM_DT)
    common["wpack"] = wpack.astype(np_mm)
    biases = np.zeros((128, N_BIAS_COLS), dtype=f32)
    for l, (ind, hid) in enumerate(LAYER_DIMS):
        n_full, r, stride, ngroups, cpg = _tail_spec(hid)
        bmats = {
            "f1": inputs[f"bf1_{l}"],
            "f2": inputs[f"bf2_{l}"],
            "t": inputs[f"btb_{l}"] - inputs[f"bta_{l}"],
        }
        for mat, b in bmats.items():
            for mi in range(n_full):
                biases[:, BIAS_COLS[(l, mat, "full", mi)]] = b[
                    mi * 128 : (mi + 1) * 128
                ]
            if r:
                col = BIAS_COLS[(l, mat, "tail")]
                for g in range(ngroups):
                    biases[g * stride : g * stride + r, col] = b[
                        n_full * 128 : n_full * 128 + r
                    ]
    common["biases"] = biases

    xT = np.zeros((K0_PAD, BATCH), dtype=np_mm)
    xT[:INPUT_DIM] = np.asarray(inputs["x"], dtype=f32).T.astype(np_mm)
    in_maps = []
    for c in range(N_CORES):
        m = dict(common)
        m["xT"] = np.ascontiguousarray(xT[:, c * B_CORE : (c + 1) * B_CORE])
        in_maps.append(m)
    return in_maps


def _unpack_out(outT_core):
    """[128, B_CORE//2] stacked -> [B_CORE, 64].

    Per slice s, column block [:, s*1024:(s+1)*1024]: rows 64g..64g+64
    hold batch chunks (2g, 2g+1) of that slice at free offsets 0/512.
    """
    out = np.empty((B_CORE, MOTOR), dtype=outT_core.dtype)
    for s in range(SLICES):
        blk = outT_core[:, s * 1024 : (s + 1) * 1024]
        for c in range(NCH):
            g, f0 = c // 2, (c % 2) * 512
            rows = slice(s * G + c * 512, s * G + (c + 1) * 512)
            out[rows, :] = blk[64 * g : 64 * g + 64, f0 : f0 + 512].T
    return out


def run(inputs, trace=False, repeat=1, **kw):
    """Run on hardware; returns (out [BATCH, MOTOR] fp32, results)."""
    nc = _get_nc(repeat)
    in_maps = _prep_host_inputs(inputs)
    res = run_bass_kernel_spmd(
        nc, in_maps, core_ids=list(range(N_CORES)), trace=trace, **kw
    )
    out = np.empty((BATCH, MOTOR), dtype=np.float32)
    for c in range(N_CORES):
        out[c * B_CORE : (c + 1) * B_CORE, :] = _unpack_out(res.results[c]["outT"])
    return out, res


def kernel(**inputs) -> np.ndarray:
    out, _ = run(inputs, trace=False)
    return out



# revision 28
# speedup vs baseline: 1.6184x; 1.0328x over previous
"""CfC head (3 stacked CfC cells, seq_len=1, h0=0) on 8 TRN2 NeuronCores.

Math (per cell, zero initial hidden state, ts=1):
    ff1 = tanh(x @ (Wf1*mask)[:in] + bf1)
    ff2 = tanh(x @ (Wf2*mask)[:in] + bf2)
    s   = sigmoid(x @ (Wtb - Wta)[:in] + (btb - bta))
    out = ff1 + s * (ff2 - ff1)

Because h0 == 0 only the first in_dim rows of each weight matter, the
sparsity mask folds into the weights, and t_a/t_b fold into a single
matmul.  That is all O(params) host-side prep; the O(B) work runs on
the NeuronCores, data-parallel over the batch.

Device layout is feature-major ([feat, batch]); x is transposed on the
host so every DMA is contiguous per partition.  Per 2048-column slice,
each layer is computed as a set of "instances":
  - full 128-row M-tiles, processed in two 1024-column halves
    (PSUM tiles are [128,1024] = 2 banks, 4 pool slots -> deep
    PE/ACT overlap), and
  - the M-tail (hid % 128 = 13/51/64 rows), batch-STACKED across
    column-tile positions of the PE array: 4 (or 2) batch chunks are
    packed onto partition groups at 32/64-partition offsets via
    tile_position, so the tail costs one narrow ACT/DVE op instead of
    four wide ones, and its matmuls run concurrently on distinct PE
    column groups.
The next layer contracts over the previous outputs in place: stacked
tail rows are consumed by row-positioned (tile_position) K-tail
matmuls against weight tiles replicated at the matching partition
offsets.
"""

import numpy as np

import concourse.bass as bass
import concourse.tile as tile
from concourse import mybir
from concourse.bass_utils import run_bass_kernel_spmd

# ---------------------------------------------------------------- dims
INPUT_DIM, INTER, COMMAND, MOTOR = 74, 269, 179, 64
# L0's contraction is zero-padded 74 -> 128: the PE activity monitor
# only counts full-row (K=128) matmuls, so K=74 work runs at the cold
# 1.2 GHz clock forever.  Full-K L0 matmuls keep the PE at 2.4 GHz.
K0_PAD = 128
BATCH = 65536
N_CORES = 8
B_CORE = BATCH // N_CORES          # 8192 rows per core
G = 2048                           # batch columns per pipeline slice
NCH = G // 512                     # 512-column matmul chunks per slice
SLICES = B_CORE // G

LAYER_DIMS = [(INPUT_DIM, INTER), (INTER, COMMAND), (COMMAND, MOTOR)]
MATS = ("f1", "f2", "t")
F32 = mybir.dt.float32
F32R = mybir.dt.float32r
# matmul operand / activation storage dtype.  bf16 allows PE column
# tile_position (stacked tails) and fast weight load; PSUM accumulation
# stays fp32.  (f32r rejects column tile positions in this walrus
# build.)
MM_DT = mybir.dt.bfloat16
# lerp elementwise dtype: fp16 keeps 10 mantissa bits and runs the DVE
# tensor_tensor ops in the 2x 16-bit mode.
EW_DT = mybir.dt.float16


def _tail_spec(hid):
    """(n_full, r, stride, ngroups, cpg) for a layer's M dimension."""
    n_full = hid // 128
    r = hid - 128 * n_full
    if r == 0:
        return n_full, 0, 0, 0, 0
    stride = 32 if r <= 32 else 64
    ngroups = 128 // stride
    cpg = NCH // ngroups           # batch chunks stacked per partition group
    return n_full, r, stride, ngroups, cpg


def _instances(hid):
    # tail first: its output gates the next layer's K-tail matmuls, so
    # let its (cheap) elementwise chain run under the fulls' matmuls.
    # Each full instance covers the whole slice (all NCH chunks) in one
    # 4-bank PSUM tile -> one wide ACT per mat.
    n_full, r, stride, ngroups, cpg = _tail_spec(hid)
    out = []
    if r:
        out.append(("tail",))
    for mi in range(n_full):
        out.append(("full", mi))
    return out


# bias pack columns: one per (layer, mat, m-range); full halves share.
BIAS_COLS = {}
_c = 0
for _l, (_in, _hid) in enumerate(LAYER_DIMS):
    _nf, _r, _st, _ng, _cpg = _tail_spec(_hid)
    for _mat in MATS:
        for _mi in range(_nf):
            BIAS_COLS[(_l, _mat, "full", _mi)] = _c
            _c += 1
        if _r:
            BIAS_COLS[(_l, _mat, "tail")] = _c
            _c += 1
N_BIAS_COLS = _c


def _mranges(l):
    # output m-ranges; tail m-size padded to the group stride with zero
    # columns so stacked PSUM groups cover every partition
    nf, r, st, ng, cpg = _tail_spec(LAYER_DIMS[l][1])
    out = [("full", mi, mi * 128, 128) for mi in range(nf)]
    if r:
        out.append(("tail", None, nf * 128, st))
    return out


def _in_kparts(l):
    if l == 0:
        return [("main", 0, 0, K0_PAD)]
    nf, r, st, ng, cpg = _tail_spec(LAYER_DIMS[l - 1][1])
    parts = [("main", ki, ki * 128, 128) for ki in range(nf)]
    if r:
        parts.append(("ktail", None, nf * 128, r))
    return parts


def _wpack_layout():
    # all lhsT tiles as column blocks of one [128, NW] array; k-tail
    # blocks carry weight rows replicated at each stacked group base
    cols = {}
    c = 0
    for l in range(len(LAYER_DIMS)):
        for mat in MATS:
            for mkind, mi, m0, msz in _mranges(l):
                for kkind, ki, k0, ksz in _in_kparts(l):
                    cols[(l, mat, mkind, mi, kkind, ki)] = (c, msz)
                    c += msz
    return cols, c


WPACK_COLS, NW = _wpack_layout()
# L0's weight columns come first in the pack; they ship as a separate
# small tensor so the first matmul only waits for ~220KB of weights
# instead of the full pack behind three xt prefetches.
W0N = max(c0 + msz for (l, *_), (c0, msz) in WPACK_COLS.items() if l == 0)
assert all((c0 >= W0N) == (l > 0) for (l, *_), (c0, msz) in WPACK_COLS.items())


# ---------------------------------------------- walrus sync-wait workaround
def _split_multi_waits(nc):
    """This walrus build accepts only ONE sync-wait command per
    instruction.  Tile attaches one wait per outstanding proc, so after
    scheduling, hoist every excess wait onto a single-wait NOP emitted
    just before the instruction on the same engine (engine queues are
    in-order, so the waits still all complete before it executes)."""
    import bass_rust as _br

    for fn in nc.m.functions:
        for blk in fn.blocks:
            out = []
            changed = False
            for inst in blk.instructions:
                si = inst.sync_info
                if si is not None and len(si.on_wait) > 1:
                    waits = list(si.on_wait)
                    for j, w in enumerate(waits[:-1]):
                        carrier = mybir.InstNoOp(
                            name=f"{inst.name}-sw{j}", engine=inst.engine
                        )
                        carrier.sync_info = _br.SyncInfo(on_wait=[w], on_update=[])
                        out.append(carrier)
                    inst.sync_info = _br.SyncInfo(
                        on_wait=[waits[-1]], on_update=list(si.on_update)
                    )
                    changed = True
                out.append(inst)
            if changed:
                blk.instructions = out
    return nc


# ---------------------------------------------------------------- device
class _LayerOut:
    """Feature-major activation of one layer for one slice.

    fulls: mi -> [128, 512*NCH] tile (feature rows mi*128..+128, all
           batch chunks of the slice).
    tail:  [128, 512*cpg] tile; partition group g holds feature rows
           n_full*128..+r for batch chunks g*cpg..(g+1)*cpg.
    """

    def __init__(self, hid):
        self.hid = hid
        self.n_full, self.r, self.stride, self.ngroups, self.cpg = _tail_spec(hid)
        self.fulls = {}
        self.tail = None

    def kparts(self):
        parts = [("main", ki, ki * 128, 128) for ki in range(self.n_full)]
        if self.r:
            parts.append(("ktail", None, self.n_full * 128, self.r))
        return parts

    def rhs(self, kind, ki, c):
        """(ap, row_pos) of this output as contraction input, chunk c."""
        if kind == "main":
            t = self.fulls[ki]
            return t[:, c * 512 : (c + 1) * 512], 0
        g = c // self.cpg
        p0 = self.stride * g
        f0 = (c % self.cpg) * 512
        return self.tail[p0 : p0 + self.r, f0 : f0 + 512], p0


def _build_nc(repeat=1):
    nc = bass.Bass(target_bir_lowering=False)

    xT = nc.dram_tensor("xT", [K0_PAD, B_CORE], MM_DT, kind="ExternalInput")
    wpack0_dram = nc.dram_tensor("wpack0", [128, W0N], MM_DT, kind="ExternalInput")
    wpack12_dram = nc.dram_tensor(
        "wpack12", [128, NW - W0N], MM_DT, kind="ExternalInput"
    )
    bias_dram = nc.dram_tensor("biases", [128, N_BIAS_COLS], F32, kind="ExternalInput")
    # stacked output layout; host unpacks (see _unpack_out)
    outT = nc.dram_tensor("outT", [128, B_CORE // 2], F32, kind="ExternalOutput")

    TANH = mybir.ActivationFunctionType.Tanh
    SIGM = mybir.ActivationFunctionType.Sigmoid

    with tile.TileContext(nc) as tc:
        with (
            tc.tile_pool(name="consts", bufs=1) as consts,
            tc.tile_pool(name="xt", bufs=3) as xt_pool,
            tc.tile_pool(name="act", bufs=2) as act_pool,
            tc.tile_pool(name="ff", bufs=4) as ff_pool,
            # two 4-bank [128, 2048] psum tiles: PE fills one while ACT
            # drains the other; wide tiles quarter the ACT instruction
            # count and shrink layer-boundary fill bubbles
            tc.tile_pool(name="ps", bufs=2, space="PSUM") as ps_pool,
        ):
            # L0 weights go FIRST on the SP ring (ahead of the xt
            # streams) so the first matmul's inputs land in ~2us; the
            # L1/L2 weights and biases ride the ACT ring concurrently
            wpack0_sb = consts.tile([128, W0N], MM_DT, tag="wpack0")
            nc.sync.dma_start(out=wpack0_sb[:], in_=wpack0_dram[:])
            bias_sb = consts.tile([128, N_BIAS_COLS], F32, tag="bias")
            nc.scalar.dma_start(out=bias_sb[:], in_=bias_dram[:])
            wpack12_sb = consts.tile([128, NW - W0N], MM_DT, tag="wpack12")
            nc.scalar.dma_start(out=wpack12_sb[:], in_=wpack12_dram[:])

            def wsb(c0, width):
                """SBUF AP for wpack columns [c0, c0+width)."""
                if c0 < W0N:
                    return wpack0_sb[:, c0 : c0 + width]
                return wpack12_sb[:, c0 - W0N : c0 - W0N + width]

            # PE warm-up: the HAM activity monitor only registers
            # full-row (K=128) matmul activity, so a kernel that opens
            # with K=74 L0 matmuls runs its entire span at the cold
            # 1.2 GHz clock.  A short block of full-K zero matmuls
            # (overlapping the initial DMA wait) trips the un-throttle;
            # the real work then issues at 2.4 GHz, and partial-K
            # activity keeps the PE from re-throttling.
            # No explicit PE warm-up block: with L0 padded to K=128 the
            # first real matmuls trip the HAM un-throttle themselves
            # (~8 cold MMs, one-time ~2.5us) — cheaper than queueing a
            # dummy block ahead of them.
            # Dummy activation: forces the ~1.3us ACT table load to run
            # during the DMA wait instead of before the first real
            # activation (tile never read).
            act_warm = consts.tile([128, 8], F32, tag="actwarm")
            nc.scalar.activation(
                out=act_warm[:], in_=bias_sb[:, 0:8], func=TANH
            )

            # ---- per-slice pipeline (repeat>1 reruns the same work for
            # differential wall-clock timing; outputs just overwritten)
            def make_xt_in(s):
                c0 = s * G
                xt = xt_pool.tile([K0_PAD, G], MM_DT, tag="xt")
                nc.sync.dma_start(out=xt[:], in_=xT[:, c0 : c0 + G])

                class _XtIn:
                    @staticmethod
                    def kparts():
                        return [("main", 0, 0, K0_PAD)]

                    @staticmethod
                    def rhs(kind, ki, c):
                        return xt[:, c * 512 : (c + 1) * 512], 0

                return _XtIn

            if True:

                def layer_tasks(l, lin, out_dtype=MM_DT):
                    """Return (lo, thunks): one thunk per instance;
                    running a thunk emits that instance's IR and files
                    its output tile into lo."""
                    lo = _LayerOut(LAYER_DIMS[l][1])
                    kps = lin.kparts()

                    def instance(inst):
                        if inst[0] == "full":
                            _, mi = inst
                            P, FF = 128, 512 * NCH
                            chunks = list(range(NCH))
                            mkind, mmi = "full", mi

                            def region(c):
                                f0 = c * 512
                                return slice(0, 128), slice(f0, f0 + 512), 0
                        else:
                            P = 128
                            FF = 512 * lo.cpg
                            # alternate column groups so adjacent matmuls
                            # land on distinct PE column tiles
                            chunks = sorted(
                                range(NCH), key=lambda c: (c % lo.cpg, c)
                            )
                            mkind, mmi = "tail", None

                            def region(c):
                                g = c // lo.cpg
                                p0 = lo.stride * g
                                f0 = (c % lo.cpg) * 512
                                return (
                                    slice(p0, p0 + lo.stride),
                                    slice(f0, f0 + 512),
                                    p0,
                                )

                        ff = {}
                        for mat in MATS:
                            ps = ps_pool.tile([P, FF], F32, tag="ps")
                            # kpart-major: one stationary-weight load
                            # serves all chunks; each chunk's bank still
                            # sees its kpi==0 matmul first
                            for kpi, (kkind, ki, k0, ksz) in enumerate(kps):
                                wc0, wmsz = WPACK_COLS[
                                    (l, mat, mkind, mmi, kkind, ki)
                                ]
                                for c in chunks:
                                    psl, fsl, colp = region(c)
                                    rhs_ap, rowp = lin.rhs(kkind, ki, c)
                                    lhsT = wsb(wc0, wmsz)[rowp : rowp + ksz, :]
                                    nc.tensor.matmul(
                                        ps[psl, fsl],
                                        lhsT,
                                        rhs_ap,
                                        start=(kpi == 0),
                                        stop=(kpi == len(kps) - 1),
                                        tile_position=(rowp, colp),
                                    )
                            f = ff_pool.tile([P, FF], EW_DT, tag=f"ff_{mat}")
                            bcol = BIAS_COLS[
                                (l, mat, "full", inst[1])
                                if inst[0] == "full"
                                else (l, mat, "tail")
                            ]
                            nc.scalar.activation(
                                out=f[:],
                                in_=ps[:],
                                func=SIGM if mat == "t" else TANH,
                                bias=bias_sb[:P, bcol : bcol + 1],
                            )
                            ff[mat] = f
                        # out = ff1 + s*(ff2-ff1); 16-bit DVE chain
                        d = ff_pool.tile([P, FF], EW_DT, tag="d")
                        nc.vector.tensor_sub(d[:], ff["f2"][:], ff["f1"][:])
                        nc.vector.tensor_mul(ff["f2"][:], ff["t"][:], d[:])
                        tag = (
                            f"o{l}_{inst[1]}"
                            if inst[0] == "full"
                            else f"o{l}_tail"
                        )
                        o = act_pool.tile([P, FF], out_dtype, tag=tag)
                        nc.vector.tensor_add(o[:], ff["f1"][:], ff["f2"][:])
                        return o

                    def run_inst(inst):
                        o = instance(inst)
                        if inst[0] == "full":
                            lo.fulls[inst[1]] = o
                        else:
                            lo.tail = o

                    thunks = [
                        (lambda inst=inst: run_inst(inst))
                        for inst in _instances(lo.hid)
                    ]
                    return lo, thunks

            # process slices in pairs, layer-major: dense same-layer
            # matmul streams keep the PE activity monitor warm (mixing
            # layers dilutes the full-K density and re-throttles the
            # PE); each layer's fill-latency overlaps the sibling
            # slice's dense work
            PAIR = 2
            total = SLICES * repeat

            def run_layer(l, lin, out_dtype=MM_DT):
                lo, thunks = layer_tasks(l, lin, out_dtype)
                for th in thunks:
                    th()
                return lo

            for pr in range(0, total, PAIR):
                sl = [(pr + j) % SLICES for j in range(min(PAIR, total - pr))]
                outs = [make_xt_in(s) for s in sl]
                outs = [run_layer(0, x) for x in outs]
                outs = [run_layer(1, o) for o in outs]
                outs = [run_layer(2, o, out_dtype=F32) for o in outs]
                for s, o2 in zip(sl, outs):
                    # L2 output is a single stacked [128, 1024] tile
                    nc.sync.dma_start(
                        out=outT[:, s * 1024 : (s + 1) * 1024], in_=o2.tail[:]
                    )

    return nc


_NC_CACHE = {}


def _get_nc(repeat=1):
    if repeat not in _NC_CACHE:
        _NC_CACHE[repeat] = _split_multi_waits(_build_nc(repeat))
    return _NC_CACHE[repeat]


# ------------------------------------------------------------------ host
def _prep_host_inputs(inputs):
    """Fold masks / t-diff, pack biases, shard x.  Returns per-core maps."""
    f32 = np.float32
    common = {}
    folded = {}
    for l, (ind, hid) in enumerate(LAYER_DIMS):
        m = inputs[f"mask_{l}"][:ind].astype(f32)
        folded[(l, "f1")] = (inputs[f"Wf1_{l}"][:ind] * m).astype(f32)
        folded[(l, "f2")] = (inputs[f"Wf2_{l}"][:ind] * m).astype(f32)
        folded[(l, "t")] = (
            inputs[f"Wtb_{l}"][:ind] - inputs[f"Wta_{l}"][:ind]
        ).astype(f32)
    for mat in MATS:  # zero-pad L0 contraction rows 74..127
        W = folded[(0, mat)]
        folded[(0, mat)] = np.concatenate(
            [W, np.zeros((K0_PAD - W.shape[0], W.shape[1]), f32)], axis=0
        )
    wpack = np.zeros((128, NW), dtype=f32)
    for (l, mat, mkind, mi, kkind, ki), (c0, msz) in WPACK_COLS.items():
        W = folded[(l, mat)]
        _, hid = LAYER_DIMS[l]
        m0 = mi * 128 if mkind == "full" else (hid // 128) * 128
        rm = min(msz, hid - m0)
        kp = [p for p in _in_kparts(l) if p[0] == kkind and p[1] == ki][0]
        _, _, k0, ksz = kp
        if kkind == "ktail":
            pnf, pr, pst, png, pcpg = _tail_spec(LAYER_DIMS[l - 1][1])
            for g in range(png):
                wpack[pst * g : pst * g + ksz, c0 : c0 + rm] = W[
                    k0 : k0 + ksz, m0 : m0 + rm
                ]
        else:
            wpack[:ksz, c0 : c0 + rm] = W[k0 : k0 + ksz, m0 : m0 + rm]
    import ml_dtypes
    np_mm = mybir.dt.np(MM_DT)
    wpack_mm = wpack.astype(np_mm)
    common["wpack0"] = np.ascontiguousarray(wpack_mm[:, :W0N])
    common["wpack12"] = np.ascontiguousarray(wpack_mm[:, W0N:])
    biases = np.zeros((128, N_BIAS_COLS), dtype=f32)
    for l, (ind, hid) in enumerate(LAYER_DIMS):
        n_full, r, stride, ngroups, cpg = _tail_spec(hid)
        bmats = {
            "f1": inputs[f"bf1_{l}"],
            "f2": inputs[f"bf2_{l}"],
            "t": inputs[f"btb_{l}"] - inputs[f"bta_{l}"],
        }
        for mat, b in bmats.items():
            for mi in range(n_full):
                biases[:, BIAS_COLS[(l, mat, "full", mi)]] = b[
                    mi * 128 : (mi + 1) * 128
                ]
            if r:
                col = BIAS_COLS[(l, mat, "tail")]
                for g in range(ngroups):
                    biases[g * stride : g * stride + r, col] = b[
                        n_full * 128 : n_full * 128 + r
                    ]
    common["biases"] = biases

    xT = np.zeros((K0_PAD, BATCH), dtype=np_mm)
    xT[:INPUT_DIM] = np.asarray(inputs["x"], dtype=f32).T.astype(np_mm)
    in_maps = []
    for c in range(N_CORES):
        m = dict(common)
        m["xT"] = np.ascontiguousarray(xT[:, c * B_CORE : (c + 1) * B_CORE])
        in_maps.append(m)
    return in_maps


def _unpack_out(outT_core):
    """[128, B_CORE//2] stacked -> [B_CORE, 64].

    Per slice s, column block [:, s*1024:(s+1)*1024]: rows 64g..64g+64
    hold batch chunks (2g, 2g+1) of that slice at free offsets 0/512.
    """
    out = np.empty((B_CORE, MOTOR), dtype=outT_core.dtype)
    for s in range(SLICES):
        blk = outT_core[:, s * 1024 : (s + 1) * 1024]
        for c in range(NCH):
            g, f0 = c // 2, (c % 2) * 512
            rows = slice(s * G + c * 512, s * G + (c + 1) * 512)
            out[rows, :] = blk[64 * g : 64 * g + 64, f0 : f0 + 512].T
    return out


def run(inputs, trace=False, repeat=1, **kw):
    """Run on hardware; returns (out [BATCH, MOTOR] fp32, results)."""
    nc = _get_nc(repeat)
    in_maps = _prep_host_inputs(inputs)
    res = run_bass_kernel_spmd(
        nc, in_maps, core_ids=list(range(N_CORES)), trace=trace, **kw
    )
    out = np.empty((BATCH, MOTOR), dtype=np.float32)
    for c in range(N_CORES):
        out[c * B_CORE : (c + 1) * B_CORE, :] = _unpack_out(res.results[c]["outT"])
    return out, res


def kernel(**inputs) -> np.ndarray:
    out, _ = run(inputs, trace=False)
    return out

